# revision 48
# baseline (speedup 1.0000x reference)
"""Trainium2 Bass kernel for nn_CBSA_45389214384209 (sparse_attention).

Reference computation (per batch element b of 8):
  x_seq = x[b].T                      # [4096, 256]   (x[b] is [256, 4096])
  proj  = x_seq @ W_proj              # [4096, 512]
  rep   = avgpool8x8(proj)            # [64, 512]  == avgpool8x8(x_seq) @ W_proj
  per head h (8 heads, dh=64):
    S    = rep_h @ proj_h.T * scale   # [64, 4096]
    P    = softmax(S)                 # [64, 4096]
    rd   = P @ proj_h                 # [64, 64]
    rep2 = rep_h + step_rep[h] * rd
    P2   = softmax(rep2 @ rep2.T * scale)
    xd2  = step_x[h] * (P2 @ rep2)    # [64, 64]
    xdT  = xd2.T @ P                  # [64, 4096]  (back-projection, transposed)
  out[b] = W_out.T @ concat_h(xdT) + b_out[:, None]   # [256, 4096]

Sharding: pure data parallel - one batch element per NeuronCore (8 cores).

Layout strategy: projections are computed "transposed" (feature dim on
partitions) straight from the native DRAM layout of x; the final
x_outT[c, t] is exactly the DRAM layout of the output.  Head pairs are
packed on partitions (rows 0-63 = even head, 64-127 = odd head) with
block-diagonal stationary tiles so one 128-wide matmul serves two heads.
P^T (needed by the rep_delta contraction over tokens) is produced by the
DMA xbar transpose engine -- the PE never transposes the big P tiles.
rep_delta is accumulated in [query, dim] orientation so every softmax
normalization is a per-partition scalar multiply.  exp(S2) is symmetric,
which lets stage 2 skip the P2 transpose.  The output bias is folded into
the last matmul as a K=1 rank-1 update and the result is DMA'd to DRAM
directly from PSUM.
"""

import os
import sys

import numpy as np

for _p in ("/opt/trn_rl_repo", os.path.expanduser("~/.axon_site/_ro/trn_rl_repo")):
    if os.path.isdir(_p) and _p not in sys.path:
        sys.path.insert(0, _p)

import concourse.bass as bass
import concourse.tile as tile
from concourse import bacc, mybir
from concourse.bass import ds, ts
from concourse.masks import make_identity

F32 = mybir.dt.float32
F32R = mybir.dt.float32r
BF16 = mybir.dt.bfloat16
AX = mybir.AxisListType
ALU = mybir.AluOpType
ACTF = mybir.ActivationFunctionType

B = 8
C = 256          # model dim
T = 4096         # tokens (64x64 grid)
INNER = 512
HEADS = 8
DH = 64
NB = 64          # pooled tokens (8x8 grid)
SCALE = DH ** -0.5
NPAIR = 4        # head pairs
NCHUNK = 8       # 512-wide token chunks
NTT = 32         # 128-wide token tiles

CFG = {
    "dmat": True,             # DMA-xbar transpose for P^T (else PE transposes)
    "bf16_pass": True,        # bf16 projection passes (FWL weight loads)
}


def build_module(cfg=CFG):
    nc = bacc.Bacc("TRN2", debug=False)

    io_dt = F32 if cfg["bf16_pass"] else F32R
    x = nc.dram_tensor("x", [C, T], io_dt, kind="ExternalInput").ap()
    wp = nc.dram_tensor("w_proj", [C, INNER], io_dt, kind="ExternalInput").ap()
    wo = nc.dram_tensor("w_out", [INNER, C], F32, kind="ExternalInput").ap()
    bo = nc.dram_tensor("b_out", [C], F32R, kind="ExternalInput").ap()
    srep = nc.dram_tensor("s_rep", [HEADS], F32, kind="ExternalInput").ap()
    sx = nc.dram_tensor("s_x", [HEADS], F32, kind="ExternalInput").ap()
    pmask = nc.dram_tensor(
        "pool_mask", [128, 8, NB], BF16, kind="ExternalInput"
    ).ap()
    out = nc.dram_tensor("out", [C, T], F32, kind="ExternalOutput").ap()

    with tile.TileContext(nc) as tc:
        _body(tc, cfg, x, wp, wo, bo, srep, sx, pmask, out)
    nc.compile()
    return nc


def _body(tc, cfg, x, wp, wo, bo, srep, sx, pmask, out):
    nc = tc.nc

    x_r = x.rearrange("(o p) t -> p o t", p=128)      # [128, 2, 4096]
    out_r = out.rearrange("(o p) t -> p o t", p=128)  # [128, 2, 4096]

    # ---- pools (SBUF pools stack-nested: alloc order == reverse release) --
    consts = tc.alloc_tile_pool(name="consts", bufs=1)
    stats = tc.alloc_tile_pool(name="stats", bufs=2)
    xdtp = tc.alloc_tile_pool(name="xdtp", bufs=2)       # xd2_bd + xdT chunks
    pp = tc.alloc_tile_pool(name="pp", bufs=1)           # P (attn) tiles
    ptp = tc.alloc_tile_pool(name="ptp", bufs=1)         # P^T tiles
    st2 = tc.alloc_tile_pool(name="st2", bufs=2)         # stage-2 temps
    prp = tc.alloc_tile_pool(name="prp", bufs=1)         # proj (bf16)
    pTp = tc.alloc_tile_pool(name="pTp", bufs=1)         # projT (bf16)
    xp = tc.alloc_tile_pool(name="xp", bufs=1)           # x (f32r)

    # PSUM tags: mm [128,2,512] x2 (4 banks) + outp x1 (2) + acc x1 + b3s x1.
    psum = tc.alloc_tile_pool(name="psum", bufs=1, space="PSUM")

    # ---- constants / weights (direct f32r DMA: same bits as f32) ---------
    # DMA issue order on the sync queue matters: wp + x chunks first (they
    # gate pass 1); everything needed later is issued after the x stream.
    ident_bf = consts.tile([128, 128], BF16, name="ident_bf")
    make_identity(nc, ident_bf)
    ident_f = consts.tile([128, 128], F32, name="ident_f")
    make_identity(nc, ident_f)

    P_DT = BF16 if cfg["bf16_pass"] else F32R
    wp_r = consts.tile([128, 2, INNER], P_DT, name="wp_r")
    if cfg["bf16_pass"]:
        nc.gpsimd.dma_start(wp_r, wp.rearrange("(o p) i -> p o i", p=128))
    else:
        nc.sync.dma_start(wp_r, wp.rearrange("(o p) i -> p o i", p=128))
    pmask_sb = consts.tile([128, 8, NB], BF16, name="pmask_sb")
    nc.sync.dma_start(pmask_sb, pmask)

    # ---- fused pass over x: projT (bf16) + proj (bf16), per 512-chunk ----
    # x streams through a 2-deep window; both projection orientations are
    # produced from the same resident chunk.
    projT = pTp.tile([128, 4, T], BF16, name="projT")
    proj_bf = prp.tile([128, NTT, INNER], BF16, name="proj_bf")
    # rep[q, d] = pool_mask^T @ proj accumulates on the PE inside the same
    # loop: pool_mask[:, br, :] has 1/64 at column (8*br + wb) for block-row
    # br; each 128-token tile lies in exactly one block-row (br = m//4).
    rep_pst = psum.tile([128, 512], F32, name="rep_pst", tag="outp", bufs=2)
    rep_ps = rep_pst[0:64, :]
    for j in range(NCHUNK):
        x_c = xp.tile([128, 2, 512], P_DT, name="x_c", tag="xc", bufs=2)
        if cfg["bf16_pass"]:
            nc.gpsimd.dma_start(x_c, x_r[:, :, ts(j, 512)])
        else:
            nc.sync.dma_start(x_c, x_r[:, :, ts(j, 512)])
        for gp in range(2):
            pt_ps = psum.tile([128, 2, 512], F32, name="pt_ps", tag="mm", bufs=2)
            for gi in range(2):
                g = 2 * gp + gi
                for o in range(2):
                    nc.tensor.matmul(
                        pt_ps[:, gi, :], wp_r[:, o, ts(g, 128)], x_c[:, o, :],
                        start=(o == 0), stop=(o == 1),
                    )
            if gp == 0:
                nc.scalar.copy(projT[:, ds(0, 2), ts(j, 512)], pt_ps)
            else:
                nc.vector.tensor_copy(projT[:, ds(2, 2), ts(j, 512)], pt_ps)
        for mp in range(2):
            pr_ps = psum.tile([128, 2, 512], F32, name="pr_ps", tag="mm", bufs=2)
            for mi in range(2):
                k = 2 * mp + mi
                for o in range(2):
                    nc.tensor.matmul(
                        pr_ps[:, mi, :], x_c[:, o, ts(k, 128)], wp_r[:, o, :],
                        start=(o == 0), stop=(o == 1),
                    )
            if mp == 0:
                nc.vector.tensor_copy(proj_bf[:, ds(4 * j, 2), :], pr_ps)
            else:
                nc.scalar.copy(proj_bf[:, ds(4 * j + 2, 2), :], pr_ps)
        # pool-MMs run one chunk behind so they never wait on the proj
        # copies of the chunk the PE just produced
        for k in range(4):
            if j == 0:
                break
            m = 4 * (j - 1) + k
            nc.tensor.matmul(
                rep_ps, pmask_sb[:, m // 4, :], proj_bf[:, m, :],
                start=(m == 0), stop=False,
            )
    for k in range(4):
        m = 4 * (NCHUNK - 1) + k
        nc.tensor.matmul(
            rep_ps, pmask_sb[:, m // 4, :], proj_bf[:, m, :],
            start=False, stop=(m == NTT - 1),
        )
    xp.release()

    # late-needed constants, issued behind the x stream
    wo_bf = consts.tile([128, 4, C], BF16, name="wo_bf")
    nc.gpsimd.dma_start(wo_bf, wo.rearrange("(g p) c -> p g c", p=128))
    bo_sb = consts.tile([1, 2, C // 2], F32R, name="bo_sb")
    nc.sync.dma_start(
        bo_sb,
        bass.AP(tensor=bo.tensor, offset=bo.offset,
                ap=[[0, 1], [C // 2, 2], [1, C // 2]]),
    )
    pmask_sb = consts.tile([128, 8, NB], BF16, name="pmask_sb")
    nc.sync.dma_start(pmask_sb, pmask)
    ones_f = consts.tile([1, 512], F32, name="ones_f")
    nc.vector.memset(ones_f, 1.0)
    ones_row = consts.tile([1, 512], F32R, name="ones_row")
    nc.vector.tensor_copy(ones_row, ones_f)

    # step_rep / step_x: column p holds step[2p] on partitions 0-63 and
    # step[2p+1] on partitions 64-127 (per-q-partition scalars).
    srep_ld = consts.tile([128, HEADS], F32, name="srep_ld")
    sx_ld = consts.tile([128, HEADS], F32, name="sx_ld")
    srep_bc = consts.tile([128, NPAIR], F32, name="srep_bc")
    sx_bc = consts.tile([128, NPAIR], F32, name="sx_bc")
    for st_dram, st_ld, st_bc in ((srep, srep_ld, srep_bc), (sx, sx_ld, sx_bc)):
        bcast = bass.AP(
            tensor=st_dram.tensor, offset=st_dram.offset,
            ap=[[0, 128], [st_dram.ap[0][0], HEADS]],
        )
        nc.sync.dma_start(st_ld, bcast)
        st_ldv = st_ld.rearrange("p (c two) -> p c two", two=2)
        for half in range(2):
            rows = slice(64 * half, 64 * half + 64)
            nc.vector.tensor_copy(st_bc[rows, :], st_ldv[rows, :, half])

    # ---- rep staging: repT_bd + rep_q from the pooled rep ----------------
    rep_sb = consts.tile([64, INNER], F32, name="rep_sb")
    nc.scalar.copy(rep_sb, rep_ps)
    # move a copy of rep onto partitions 64-127 (for the qB half of rep_q)
    qb_ps = psum.tile([128, 512], F32, name="qb_ps", tag="outp", bufs=2)
    nc.tensor.matmul(
        qb_ps[64:128, :], ident_f[0:64, 0:64], rep_sb, start=True, stop=True,
    )
    # repT_bd: scaled block-diagonal [[sA, 0], [0, sB]] per pair (one K=128
    # matmul computes both heads' scores; zeros kill the cross terms), and
    # rep_q[p] = [128 (qA|qB), 64 d] for the stage-2 update.
    repT_bd = consts.tile([128, NPAIR, 128], BF16, name="repT_bd")
    nc.vector.memset(repT_bd, 0.0)
    rep_q = consts.tile([128, NPAIR, NB], F32, name="rep_q")
    for g in range(4):
        rT_ps = psum.tile([128, NB], F32, name="rT_ps", tag="acc", bufs=1)
        nc.tensor.transpose(
            rT_ps, rep_sb[:, ds(128 * g, 128)], ident_f[0:64, 0:64]
        )
        for h in range(2):
            rows = slice(64 * h, 64 * h + 64)
            nc.vector.tensor_scalar_mul(
                repT_bd[rows, g, ds(64 * h, 64)], rT_ps[rows, :], SCALE
            )
        nc.vector.tensor_copy(rep_q[0:64, g, :], rep_sb[:, ds(128 * g, 64)])
        nc.vector.tensor_copy(
            rep_q[64:128, g, :], qb_ps[64:128, ds(128 * g + 64, 64)]
        )

    # ---- attention stages, pipelined per head pair ----------------------
    # P_sb[p]: [128, T]; rows 0-63 = head 2p, rows 64-127 = head 2p+1.
    # Scores are q.k/8 with pooled queries -> bounded |s| ~< 2, so softmax
    # needs no max-subtraction; exp reads score pairs straight from PSUM.
    # P stays unnormalized; 1/Z folds into rep_delta (per-partition) and
    # into the back-projection stationary (rz*rz2*step_x on xd2).
    p_tiles = [
        pp.tile([128, T], BF16, name=f"p{p}", tag=f"p{p}") for p in range(NPAIR)
    ]
    xd2_tiles = []
    xdt_tiles = [[], []]
    pt_list = [None] * NPAIR
    rz_list = [None] * NPAIR

    def _score_phase(p):
        # S + exp + P^T for pair p.  Sub-DMA-transposes chase the exp stream
        # so pt is ready ~one group after the last exp, not 3.6us later.
        zpart = stats.tile([128, 4], F32, name="zpart", tag="zpart")
        pt_t = ptp.tile([128, NTT, 128], BF16, name=f"pt{p}", tag="pt", bufs=4)
        for jp in range(4):
            s_ps = psum.tile([128, 2, 512], F32, name="s_ps", tag="mm", bufs=2)
            for ji in range(2):
                nc.tensor.matmul(
                    s_ps[:, ji, :], repT_bd[:, p, :],
                    projT[:, p, ts(2 * jp + ji, 512)],
                    start=True, stop=True,
                )
            nc.scalar.activation(
                out=p_tiles[p][:, ts(jp, 1024)].rearrange(
                    "p (a b) -> p a b", a=2),
                in_=s_ps, func=ACTF.Exp,
                bias=0.0, scale=1.0, accum_out=zpart[:, jp : jp + 1],
            )
            if cfg["dmat"]:
                nc.sync.dma_start_transpose(
                    pt_t[:, ds(8 * jp, 8), :],
                    p_tiles[p][:, ts(jp, 1024)],
                )
            else:
                for mq in range(2):
                    tp_ps = psum.tile(
                        [128, 4, 128], BF16, name="tp_ps", tag="b3s", bufs=1)
                    for c4 in range(4):
                        nc.tensor.transpose(
                            tp_ps[:, c4, :],
                            p_tiles[p][:, ts(8 * jp + 4 * mq + c4, 128)],
                            ident_bf)
                    nc.vector.tensor_copy(
                        pt_t[:, ds(8 * jp + 4 * mq, 4), :], tp_ps)
        zsum = stats.tile([128, 1], F32, name="zsum", tag="zsum")
        nc.vector.reduce_sum(zsum, zpart, axis=AX.X)
        rz = stats.tile([128, 1], F32, name="rz", tag=f"rzp{p}", bufs=1)
        nc.vector.reciprocal(rz, zsum)
        pt_list[p] = pt_t
        rz_list[p] = rz
        if p == NPAIR - 1:
            pTp.release()

    def _delta_phase(p):
        rz = rz_list[p]
        # rep_delta [q, d]: av[q, d'] = sum_t P[q,t] proj[t,d'] accumulated
        # over 32 token tiles; diagonal 64x64 blocks are the two heads'.
        av_ps = psum.tile([128, 128], F32, name="av_ps", tag="acc", bufs=1)
        for m in range(NTT):
            nc.tensor.matmul(
                av_ps, pt_list[p][:, m, :], proj_bf[:, m, ds(128 * p, 128)],
                start=(m == 0), stop=(m == NTT - 1),
            )
        # rep2 = rep_q + (step_rep * rz) * rep_delta   [128 q-pair, 64 d]
        srz = stats.tile([128, 1], F32, name="srz", tag="srz")
        nc.vector.tensor_mul(srz, rz, srep_bc[:, p : p + 1])
        rep2 = st2.tile([128, NB], BF16, name="rep2", tag="rep2")
        for h in range(2):
            rows = slice(64 * h, 64 * h + 64)
            nc.vector.scalar_tensor_tensor(
                rep2[rows, :], av_ps[rows, ds(64 * h, 64)], srz[rows, :],
                rep_q[rows, p, :], op0=ALU.mult, op1=ALU.add,
            )
        # rep2T block-diag (zeros kill cross-head terms in S2)
        r2t_ps = psum.tile([64, 128], BF16, name="r2t_ps", tag="b3s", bufs=1)
        nc.tensor.transpose(r2t_ps, rep2, ident_bf)
        r2bd = st2.tile([128, 128], BF16, name="r2bd", tag="r2bd")
        nc.vector.memset(r2bd, 0.0)
        for h in range(2):
            nc.vector.tensor_copy(
                r2bd[ds(64 * h, 64), ds(64 * h, 64)],
                r2t_ps[:, ds(64 * h, 64)],
            )
        # S2 = rep2 @ rep2^T (both heads at once); exp(SCALE*S2) on the
        # diagonal quadrants only (cross quadrants stay zero).  exp(S2) is
        # symmetric, so it serves directly as lhsT for xd2 = P2 @ rep2.
        s2_ps = psum.tile([128, 128], F32, name="s2_ps", tag="b3s", bufs=1)
        nc.tensor.matmul(s2_ps, r2bd, r2bd, start=True, stop=True)
        es = st2.tile([128, 128], BF16, name="es", tag="es")
        nc.vector.memset(es, 0.0)
        z2 = stats.tile([128, 1], F32, name="z2", tag="z2")
        for h in range(2):
            rows = slice(64 * h, 64 * h + 64)
            nc.scalar.activation(
                out=es[rows, ds(64 * h, 64)], in_=s2_ps[rows, ds(64 * h, 64)],
                func=ACTF.Exp, bias=0.0, scale=SCALE,
                accum_out=z2[rows, :],
            )
        rz2 = stats.tile([128, 1], F32, name="rz2", tag="rz2")
        nc.vector.reciprocal(rz2, z2)
        xd_ps = psum.tile([128, NB], F32, name="xd_ps", tag="b3s", bufs=1)
        nc.tensor.matmul(xd_ps, es, rep2, start=True, stop=True)
        # xd2_bd: block-diagonal stationary for the back-projection, with
        # rz (stage-1 softmax), rz2 (stage-2 softmax) and step_x folded in.
        rzsx = stats.tile([128, 1], F32, name="rzsx", tag="rzsx")
        nc.vector.tensor_mul(rzsx, rz, sx_bc[:, p : p + 1])
        nc.vector.tensor_mul(rzsx, rzsx, rz2)
        xd2_bd = xdtp.tile([128, 128], BF16, name="xd2_bd", tag=f"xd2_{p}",
                           bufs=1)
        nc.vector.memset(xd2_bd, 0.0)
        for h in range(2):
            rows = slice(64 * h, 64 * h + 64)
            nc.vector.tensor_scalar_mul(
                xd2_bd[rows, ds(64 * h, 64)], xd_ps[rows, :], rzsx[rows, :],
            )
        xd2_tiles.append(xd2_bd)

    def _bp_phase(pq):
        # back-projection for a pair-pair; pq=0 overlaps pair-3 attention.
        # pi-major order with 1024-wide moving operands: each xd2 stationary
        # is loaded once and streams 4 matmuls.
        for pi in range(2):
            for j2 in range(NCHUNK // 2):
                bp_ps = psum.tile(
                    [128, 2, 512], F32, name="bp_ps", tag="mm", bufs=2)
                for ji in range(2):
                    nc.tensor.matmul(
                        bp_ps[:, ji, :], xd2_tiles[2 * pq + pi],
                        p_tiles[2 * pq + pi][:, ts(2 * j2 + ji, 512)],
                        start=True, stop=True,
                    )
                xdt_sb = xdtp.tile(
                    [128, 2, 512], BF16, name="xdt_sb",
                    tag=f"xdt_{pq}_{pi}_{j2}", bufs=1)
                # pq=0 runs under the exp stream: keep ACT free for exps
                if pq == 0 or j2 % 2 == 0:
                    nc.vector.tensor_copy(xdt_sb, bp_ps)
                else:
                    nc.scalar.copy(xdt_sb, bp_ps)
                xdt_tiles[pq].append(xdt_sb)

    # software pipeline with a 1-pair offset: the PE fills the wait for
    # pair p's P^T (exp + DMA transpose) with pair p-1's delta/stage-2
    # work; back-projections are emitted one step after their xd2 tiles
    # complete so the in-order PE queue never waits on a stage-2 chain.
    for step in range(NPAIR + 2):
        if step < NPAIR:
            _score_phase(step)
        if 1 <= step <= NPAIR:
            _delta_phase(step - 1)
        if step >= 3 and (step - 3) % 2 == 0:
            _bp_phase((step - 3) // 2)
    prp.release()
    st2.release()
    ptp.release()
    pp.release()

    # ---- output projection, per (chunk, 128-channel group) ---------------
    for j in range(NCHUNK):
        for ct in range(2):
            op_ps = psum.tile([128, 512], F32, name="op_ps", tag="outp",
                              bufs=2)
            for g in range(4):
                nc.tensor.matmul(
                    op_ps, wo_bf[:, g, ts(ct, 128)],
                    xdt_tiles[g // 2][(g % 2) * 4 + j // 2][:, j % 2, :],
                    start=(g == 0), stop=False,
                )
            nc.tensor.matmul(
                op_ps, bo_sb[:, ct, :], ones_row, start=False, stop=True,
            )
            out_sb = xdtp.tile([128, 512], F32, name="out_sb", tag="out_sb")
            if (2 * j + ct) % 2 == 0:
                nc.scalar.copy(out_sb, op_ps)
            else:
                nc.vector.tensor_copy(out_sb, op_ps)
            nc.sync.dma_start(out_r[:, ct, ts(j, 512)], out_sb)
    xdtp.release()
    psum.release()
    stats.release()
    consts.release()


_CACHE = {}


class _Runner:
    """Builds the Bass module once and keeps a single jitted shard_map
    executable alive, so repeat kernel() calls skip retracing/relowering."""

    def __init__(self):
        import jax
        import jax.numpy as jnp
        from jax.sharding import Mesh, PartitionSpec
        from jax.experimental.shard_map import shard_map
        from concourse import bass2jax

        self.jax = jax
        nc = build_module()
        self.nc = nc
        bass2jax.install_neuronx_cc_hook()

        partition_name = (
            nc.partition_id_tensor.name if nc.partition_id_tensor else None
        )
        in_names, out_names, out_avals = [], [], []
        for alloc in nc.m.functions[0].allocations:
            if not isinstance(alloc, mybir.MemoryLocationSet):
                continue
            name = alloc.memorylocations[0].name
            if alloc.kind == "ExternalInput":
                if name != partition_name:
                    in_names.append(name)
            elif alloc.kind == "ExternalOutput":
                out_names.append(name)
                out_avals.append(
                    jax.core.ShapedArray(
                        tuple(alloc.tensor_shape), mybir.dt.np(alloc.dtype)
                    )
                )
        n_params = len(in_names)
        n_outs = len(out_avals)
        all_names = list(in_names) + list(out_names)
        if partition_name is not None:
            all_names.append(partition_name)
        self.in_names = in_names
        self.out_names = out_names
        self.out_avals = out_avals

        def _body(*args):
            operands = list(args)
            if partition_name is not None:
                operands.append(bass2jax.partition_id_tensor())
            outs = bass2jax._bass_exec_p.bind(
                *operands,
                out_avals=tuple(out_avals),
                in_names=tuple(all_names),
                out_names=tuple(out_names),
                lowering_input_output_aliases=(),
                sim_require_finite=True,
                sim_require_nnan=True,
                nc=nc,
            )
            return tuple(outs)

        self.body = _body
        devices = jax.devices()[:B]
        mesh = Mesh(np.asarray(devices), ("core",))
        donate = tuple(range(n_params, n_params + n_outs))
        self.sharded = jax.jit(
            shard_map(
                _body, mesh=mesh,
                in_specs=(PartitionSpec("core"),) * (n_params + n_outs),
                out_specs=(PartitionSpec("core"),) * n_outs,
                check_rep=False,
            ),
            donate_argnums=donate,
            keep_unused=True,
        )

    def run(self, in_maps):
        concat_in = [
            np.concatenate([m[name] for m in in_maps], axis=0)
            for name in self.in_names
        ]
        zeros = [
            np.zeros((B * a.shape[0], *a.shape[1:]), a.dtype) for a in self.out_avals
        ]
        out_arrs = self.sharded(*concat_in, *zeros)
        return [
            {
                name: np.asarray(out_arrs[i]).reshape(B, *self.out_avals[i].shape)[c]
                for i, name in enumerate(self.out_names)
            }
            for c in range(B)
        ]

    def bench(self, in_maps, reps=8, inner=72, base=8):
        """Time device-resident executions (no donation, operands staged once).

        Times jitted chains of `base` and `inner` back-to-back kernel
        executions; returns (per_exec_seconds, base_chain_seconds, results)
        with per_exec = (t_inner - t_base) / (inner - base), which amortizes
        away the per-dispatch round-trip of this axon-tunneled environment.
        """
        import time
        from jax.sharding import Mesh, PartitionSpec, NamedSharding
        from jax.experimental.shard_map import shard_map

        jax = self.jax
        devices = jax.devices()[:B]
        mesh = Mesh(np.asarray(devices), ("core",))
        sharding = NamedSharding(mesh, PartitionSpec("core"))
        n_ops = len(self.in_names) + len(self.out_avals)

        def chain(n):
            def f(*args):
                outs = []
                for _ in range(n):
                    outs.extend(self.body(*args))
                return tuple(outs)
            return f

        concat_in = [
            np.concatenate([m[name] for m in in_maps], axis=0)
            for name in self.in_names
        ]
        zeros = [
            np.zeros((B * a.shape[0], *a.shape[1:]), a.dtype) for a in self.out_avals
        ]
        staged = [jax.device_put(a, sharding) for a in concat_in + zeros]

        times = {}
        out1 = None
        for n in (base, inner):
            jfn = jax.jit(
                shard_map(
                    chain(n), mesh=mesh,
                    in_specs=(PartitionSpec("core"),) * n_ops,
                    out_specs=(PartitionSpec("core"),) * (n * len(self.out_avals)),
                    check_rep=False,
                ),
                keep_unused=True,
            )
            out = jfn(*staged)
            jax.block_until_ready(out)
            best = float("inf")
            for _ in range(reps):
                t0 = time.perf_counter()
                out = jfn(*staged)
                jax.block_until_ready(out)
                best = min(best, time.perf_counter() - t0)
            times[n] = best
            if n == base:
                out1 = out
        per_exec = (times[inner] - times[base]) / (inner - base)
        if per_exec <= 0:
            per_exec = times[inner] / inner  # noise floor: report upper bound
        results = [
            {
                name: np.asarray(out1[i]).reshape(B, *self.out_avals[i].shape)[c]
                for i, name in enumerate(self.out_names)
            }
            for c in range(B)
        ]
        return per_exec, times[base], results


def _get_runner():
    key = tuple(sorted(CFG.items()))
    if key not in _CACHE:
        _CACHE[key] = _Runner()
    return _CACHE[key]


def _pool_mask():
    import ml_dtypes

    mask = np.zeros((128, 8, NB), dtype=np.float32)
    for tp in range(128):
        wb = (tp % 64) // 8
        for br in range(8):
            mask[tp, br, 8 * br + wb] = 1.0 / 64.0
    return mask.astype(ml_dtypes.bfloat16)


def _make_in_maps(x, W_proj, step_rep, step_x, W_out, b_out):
    x = np.ascontiguousarray(np.asarray(x, dtype=np.float32))
    shared = {
        "w_proj": np.ascontiguousarray(np.asarray(W_proj, dtype=np.float32)),
        "w_out": np.ascontiguousarray(np.asarray(W_out, dtype=np.float32)),
        "b_out": np.ascontiguousarray(np.asarray(b_out, dtype=np.float32)),
        "s_rep": np.ascontiguousarray(
            np.asarray(step_rep, dtype=np.float32).reshape(HEADS)
        ),
        "s_x": np.ascontiguousarray(
            np.asarray(step_x, dtype=np.float32).reshape(HEADS)
        ),
        "pool_mask": _pool_mask(),
    }
    return [
        {"x": np.ascontiguousarray(x[b].reshape(C, T)), **shared} for b in range(B)
    ]


def kernel(x, W_proj, step_rep, step_x, W_out, b_out):
    runner = _get_runner()
    results = runner.run(_make_in_maps(x, W_proj, step_rep, step_x, W_out, b_out))
    outs = [np.asarray(results[b]["out"]).reshape(C, 64, 64) for b in range(B)]
    return np.stack(outs, axis=0)


# revision 55
# speedup vs baseline: 1.0388x; 1.0388x over previous
"""Trainium2 Bass kernel for nn_CBSA_45389214384209 (sparse_attention).

Reference computation (per batch element b of 8):
  x_seq = x[b].T                      # [4096, 256]   (x[b] is [256, 4096])
  proj  = x_seq @ W_proj              # [4096, 512]
  rep   = avgpool8x8(proj)            # [64, 512]  == avgpool8x8(x_seq) @ W_proj
  per head h (8 heads, dh=64):
    S    = rep_h @ proj_h.T * scale   # [64, 4096]
    P    = softmax(S)                 # [64, 4096]
    rd   = P @ proj_h                 # [64, 64]
    rep2 = rep_h + step_rep[h] * rd
    P2   = softmax(rep2 @ rep2.T * scale)
    xd2  = step_x[h] * (P2 @ rep2)    # [64, 64]
    xdT  = xd2.T @ P                  # [64, 4096]  (back-projection, transposed)
  out[b] = W_out.T @ concat_h(xdT) + b_out[:, None]   # [256, 4096]

Sharding: pure data parallel - one batch element per NeuronCore (8 cores).

Layout strategy: projections are computed "transposed" (feature dim on
partitions) straight from the native DRAM layout of x; the final
x_outT[c, t] is exactly the DRAM layout of the output.  Head pairs are
packed on partitions (rows 0-63 = even head, 64-127 = odd head) with
block-diagonal stationary tiles so one 128-wide matmul serves two heads.
P^T (needed by the rep_delta contraction over tokens) is produced by the
DMA xbar transpose engine -- the PE never transposes the big P tiles.
rep_delta is accumulated in [query, dim] orientation so every softmax
normalization is a per-partition scalar multiply.  exp(S2) is symmetric,
which lets stage 2 skip the P2 transpose.  The output bias is folded into
the last matmul as a K=1 rank-1 update and the result is DMA'd to DRAM
directly from PSUM.
"""

import os
import sys

import numpy as np

for _p in ("/opt/trn_rl_repo", os.path.expanduser("~/.axon_site/_ro/trn_rl_repo")):
    if os.path.isdir(_p) and _p not in sys.path:
        sys.path.insert(0, _p)

import concourse.bass as bass
import concourse.tile as tile
from concourse import bacc, mybir
from concourse.bass import ds, ts
from concourse.masks import make_identity

F32 = mybir.dt.float32
F32R = mybir.dt.float32r
BF16 = mybir.dt.bfloat16
AX = mybir.AxisListType
ALU = mybir.AluOpType
ACTF = mybir.ActivationFunctionType

B = 8
C = 256          # model dim
T = 4096         # tokens (64x64 grid)
INNER = 512
HEADS = 8
DH = 64
NB = 64          # pooled tokens (8x8 grid)
SCALE = DH ** -0.5
NPAIR = 4        # head pairs
NCHUNK = 8       # 512-wide token chunks
NTT = 32         # 128-wide token tiles

CFG = {
    "dmat": True,             # DMA-xbar transpose for P^T (else PE transposes)
    "bf16_pass": False,       # bf16 projection passes (slower: swdge casts)
}


def build_module(cfg=CFG):
    nc = bacc.Bacc("TRN2", debug=False)

    io_dt = F32 if cfg["bf16_pass"] else F32R
    x = nc.dram_tensor("x", [C, T], io_dt, kind="ExternalInput").ap()
    wp = nc.dram_tensor("w_proj", [C, INNER], io_dt, kind="ExternalInput").ap()
    wo = nc.dram_tensor("w_out", [INNER, C], F32, kind="ExternalInput").ap()
    bo = nc.dram_tensor("b_out", [C], F32R, kind="ExternalInput").ap()
    srep = nc.dram_tensor("s_rep", [HEADS], F32, kind="ExternalInput").ap()
    sx = nc.dram_tensor("s_x", [HEADS], F32, kind="ExternalInput").ap()
    pmask = nc.dram_tensor(
        "pool_mask", [128, 8, NB], BF16, kind="ExternalInput"
    ).ap()
    out = nc.dram_tensor("out", [C, T], F32, kind="ExternalOutput").ap()

    with tile.TileContext(nc) as tc:
        _body(tc, cfg, x, wp, wo, bo, srep, sx, pmask, out)
    nc.compile()
    return nc


def _body(tc, cfg, x, wp, wo, bo, srep, sx, pmask, out):
    nc = tc.nc

    x_r = x.rearrange("(o p) t -> p o t", p=128)      # [128, 2, 4096]
    out_r = out.rearrange("(o p) t -> p o t", p=128)  # [128, 2, 4096]

    # ---- pools (SBUF pools stack-nested: alloc order == reverse release) --
    consts = tc.alloc_tile_pool(name="consts", bufs=1)
    stats = tc.alloc_tile_pool(name="stats", bufs=2)
    xdtp = tc.alloc_tile_pool(name="xdtp", bufs=2)       # xd2_bd + xdT chunks
    pp = tc.alloc_tile_pool(name="pp", bufs=1)           # P (attn) tiles
    ptp = tc.alloc_tile_pool(name="ptp", bufs=1)         # P^T tiles
    st2 = tc.alloc_tile_pool(name="st2", bufs=2)         # stage-2 temps
    prp = tc.alloc_tile_pool(name="prp", bufs=1)         # proj (bf16)
    pTp = tc.alloc_tile_pool(name="pTp", bufs=1)         # projT (bf16)
    xp = tc.alloc_tile_pool(name="xp", bufs=1)           # x (f32r)

    # PSUM tags: mm [128,2,512] x2 (4 banks) + outp x1 (2) + acc x1 + b3s x1.
    psum = tc.alloc_tile_pool(name="psum", bufs=1, space="PSUM")

    # ---- constants / weights (direct f32r DMA: same bits as f32) ---------
    # DMA issue order on the sync queue matters: wp + x chunks first (they
    # gate pass 1); everything needed later is issued after the x stream.
    ident_bf = consts.tile([128, 128], BF16, name="ident_bf")
    make_identity(nc, ident_bf)
    ident_f = consts.tile([128, 128], F32, name="ident_f")
    make_identity(nc, ident_f)

    P_DT = BF16 if cfg["bf16_pass"] else F32R
    wp_r = consts.tile([128, 2, INNER], P_DT, name="wp_r")
    if cfg["bf16_pass"]:
        nc.gpsimd.dma_start(wp_r, wp.rearrange("(o p) i -> p o i", p=128))
    else:
        nc.sync.dma_start(wp_r, wp.rearrange("(o p) i -> p o i", p=128))
    pmask_sb = consts.tile([128, 8, NB], BF16, name="pmask_sb")
    nc.sync.dma_start(pmask_sb, pmask)

    # ---- fused pass over x: projT (bf16) + proj (bf16), per 512-chunk ----
    # x streams through a 2-deep window; both projection orientations are
    # produced from the same resident chunk.
    projT = pTp.tile([128, 4, T], BF16, name="projT")
    proj_bf = prp.tile([128, NTT, INNER], BF16, name="proj_bf")
    # rep[q, d] = pool_mask^T @ proj accumulates on the PE inside the same
    # loop: pool_mask[:, br, :] has 1/64 at column (8*br + wb) for block-row
    # br; each 128-token tile lies in exactly one block-row (br = m//4).
    rep_pst = psum.tile([128, 512], F32, name="rep_pst", tag="outp", bufs=2)
    rep_ps = rep_pst[0:64, :]
    for j in range(NCHUNK):
        x_c = xp.tile([128, 2, 512], P_DT, name="x_c", tag="xc", bufs=2)
        if cfg["bf16_pass"]:
            nc.gpsimd.dma_start(x_c, x_r[:, :, ts(j, 512)])
        else:
            nc.sync.dma_start(x_c, x_r[:, :, ts(j, 512)])
        for gp in range(2):
            pt_ps = psum.tile([128, 2, 512], F32, name="pt_ps", tag="mm", bufs=2)
            for gi in range(2):
                g = 2 * gp + gi
                for o in range(2):
                    nc.tensor.matmul(
                        pt_ps[:, gi, :], wp_r[:, o, ts(g, 128)], x_c[:, o, :],
                        start=(o == 0), stop=(o == 1),
                    )
            if gp == 0:
                nc.scalar.copy(projT[:, ds(0, 2), ts(j, 512)], pt_ps)
            else:
                nc.vector.tensor_copy(projT[:, ds(2, 2), ts(j, 512)], pt_ps)
        for mp in range(2):
            pr_ps = psum.tile([128, 2, 512], F32, name="pr_ps", tag="mm", bufs=2)
            for mi in range(2):
                k = 2 * mp + mi
                for o in range(2):
                    nc.tensor.matmul(
                        pr_ps[:, mi, :], x_c[:, o, ts(k, 128)], wp_r[:, o, :],
                        start=(o == 0), stop=(o == 1),
                    )
            if mp == 0:
                nc.vector.tensor_copy(proj_bf[:, ds(4 * j, 2), :], pr_ps)
            else:
                nc.scalar.copy(proj_bf[:, ds(4 * j + 2, 2), :], pr_ps)
        # pool-MMs run one chunk behind so they never wait on the proj
        # copies of the chunk the PE just produced
        for k in range(4):
            if j == 0:
                break
            m = 4 * (j - 1) + k
            nc.tensor.matmul(
                rep_ps, pmask_sb[:, m // 4, :], proj_bf[:, m, :],
                start=(m == 0), stop=False,
            )
    for k in range(4):
        m = 4 * (NCHUNK - 1) + k
        nc.tensor.matmul(
            rep_ps, pmask_sb[:, m // 4, :], proj_bf[:, m, :],
            start=False, stop=(m == NTT - 1),
        )
    xp.release()

    # late-needed constants, issued behind the x stream
    wo_bf = consts.tile([128, 4, C], BF16, name="wo_bf")
    nc.gpsimd.dma_start(wo_bf, wo.rearrange("(g p) c -> p g c", p=128))
    bo_sb = consts.tile([1, 2, C // 2], F32R, name="bo_sb")
    nc.sync.dma_start(
        bo_sb,
        bass.AP(tensor=bo.tensor, offset=bo.offset,
                ap=[[0, 1], [C // 2, 2], [1, C // 2]]),
    )
    pmask_sb = consts.tile([128, 8, NB], BF16, name="pmask_sb")
    nc.sync.dma_start(pmask_sb, pmask)
    ones_f = consts.tile([1, 512], F32, name="ones_f")
    nc.vector.memset(ones_f, 1.0)
    ones_row = consts.tile([1, 512], F32R, name="ones_row")
    nc.vector.tensor_copy(ones_row, ones_f)

    # step_rep / step_x: column p holds step[2p] on partitions 0-63 and
    # step[2p+1] on partitions 64-127 (per-q-partition scalars).
    srep_ld = consts.tile([128, HEADS], F32, name="srep_ld")
    sx_ld = consts.tile([128, HEADS], F32, name="sx_ld")
    srep_bc = consts.tile([128, NPAIR], F32, name="srep_bc")
    sx_bc = consts.tile([128, NPAIR], F32, name="sx_bc")
    for st_dram, st_ld, st_bc in ((srep, srep_ld, srep_bc), (sx, sx_ld, sx_bc)):
        bcast = bass.AP(
            tensor=st_dram.tensor, offset=st_dram.offset,
            ap=[[0, 128], [st_dram.ap[0][0], HEADS]],
        )
        nc.sync.dma_start(st_ld, bcast)
        st_ldv = st_ld.rearrange("p (c two) -> p c two", two=2)
        for half in range(2):
            rows = slice(64 * half, 64 * half + 64)
            nc.vector.tensor_copy(st_bc[rows, :], st_ldv[rows, :, half])

    # ---- rep staging: repT_bd + rep_q from the pooled rep ----------------
    rep_sb = consts.tile([64, INNER], F32, name="rep_sb")
    nc.scalar.copy(rep_sb, rep_ps)
    # move a copy of rep onto partitions 64-127 (for the qB half of rep_q)
    qb_ps = psum.tile([128, 512], F32, name="qb_ps", tag="outp", bufs=2)
    nc.tensor.matmul(
        qb_ps[64:128, :], ident_f[0:64, 0:64], rep_sb, start=True, stop=True,
    )
    # repT_bd: scaled block-diagonal [[sA, 0], [0, sB]] per pair (one K=128
    # matmul computes both heads' scores; zeros kill the cross terms), and
    # rep_q[p] = [128 (qA|qB), 64 d] for the stage-2 update.
    repT_bd = consts.tile([128, NPAIR, 128], BF16, name="repT_bd")
    nc.vector.memset(repT_bd, 0.0)
    rep_q = consts.tile([128, NPAIR, NB], F32, name="rep_q")
    for g in range(4):
        rT_ps = psum.tile([128, NB], F32, name="rT_ps", tag="acc", bufs=1)
        nc.tensor.transpose(
            rT_ps, rep_sb[:, ds(128 * g, 128)], ident_f[0:64, 0:64]
        )
        for h in range(2):
            rows = slice(64 * h, 64 * h + 64)
            nc.vector.tensor_scalar_mul(
                repT_bd[rows, g, ds(64 * h, 64)], rT_ps[rows, :], SCALE
            )
        nc.vector.tensor_copy(rep_q[0:64, g, :], rep_sb[:, ds(128 * g, 64)])
        nc.vector.tensor_copy(
            rep_q[64:128, g, :], qb_ps[64:128, ds(128 * g + 64, 64)]
        )

    # ---- attention stages, pipelined per head pair ----------------------
    # P_sb[p]: [128, T]; rows 0-63 = head 2p, rows 64-127 = head 2p+1.
    # Scores are q.k/8 with pooled queries -> bounded |s| ~< 2, so softmax
    # needs no max-subtraction; exp reads score pairs straight from PSUM.
    # P stays unnormalized; 1/Z folds into rep_delta (per-partition) and
    # into the back-projection stationary (rz*rz2*step_x on xd2).
    p_tiles = [
        pp.tile([128, T], BF16, name=f"p{p}", tag=f"p{p}") for p in range(NPAIR)
    ]
    xd2_tiles = []
    xdt_tiles = [[] for _ in range(NPAIR)]
    pt_list = [None] * NPAIR
    rz_list = [None] * NPAIR

    def _score_phase(p):
        # S + exp + P^T for pair p.  Sub-DMA-transposes chase the exp stream
        # so pt is ready ~one group after the last exp, not 3.6us later.
        zpart = stats.tile([128, 4], F32, name="zpart", tag="zpart")
        pt_t = ptp.tile([128, NTT, 128], BF16, name=f"pt{p}", tag="pt", bufs=4)
        for jp in range(4):
            s_ps = psum.tile([128, 2, 512], F32, name="s_ps", tag="mm", bufs=2)
            for ji in range(2):
                nc.tensor.matmul(
                    s_ps[:, ji, :], repT_bd[:, p, :],
                    projT[:, p, ts(2 * jp + ji, 512)],
                    start=True, stop=True,
                )
            nc.scalar.activation(
                out=p_tiles[p][:, ts(jp, 1024)].rearrange(
                    "p (a b) -> p a b", a=2),
                in_=s_ps, func=ACTF.Exp,
                bias=0.0, scale=1.0, accum_out=zpart[:, jp : jp + 1],
            )
            if cfg["dmat"]:
                nc.sync.dma_start_transpose(
                    pt_t[:, ds(8 * jp, 8), :],
                    p_tiles[p][:, ts(jp, 1024)],
                )
            else:
                for mq in range(2):
                    tp_ps = psum.tile(
                        [128, 4, 128], BF16, name="tp_ps", tag="b3s", bufs=1)
                    for c4 in range(4):
                        nc.tensor.transpose(
                            tp_ps[:, c4, :],
                            p_tiles[p][:, ts(8 * jp + 4 * mq + c4, 128)],
                            ident_bf)
                    nc.vector.tensor_copy(
                        pt_t[:, ds(8 * jp + 4 * mq, 4), :], tp_ps)
        zsum = stats.tile([128, 1], F32, name="zsum", tag="zsum")
        nc.vector.reduce_sum(zsum, zpart, axis=AX.X)
        rz = stats.tile([128, 1], F32, name="rz", tag=f"rzp{p}", bufs=1)
        nc.vector.reciprocal(rz, zsum)
        pt_list[p] = pt_t
        rz_list[p] = rz
        if p == NPAIR - 1:
            pTp.release()

    def _delta_phase(p):
        rz = rz_list[p]
        # rep_delta [q, d]: av[q, d'] = sum_t P[q,t] proj[t,d'] accumulated
        # over 32 token tiles; diagonal 64x64 blocks are the two heads'.
        av_ps = psum.tile([128, 128], F32, name="av_ps", tag="acc", bufs=1)
        for m in range(NTT):
            nc.tensor.matmul(
                av_ps, pt_list[p][:, m, :], proj_bf[:, m, ds(128 * p, 128)],
                start=(m == 0), stop=(m == NTT - 1),
            )
        # rep2 = rep_q + (step_rep * rz) * rep_delta   [128 q-pair, 64 d]
        srz = stats.tile([128, 1], F32, name="srz", tag="srz")
        nc.vector.tensor_mul(srz, rz, srep_bc[:, p : p + 1])
        rep2 = st2.tile([128, NB], BF16, name="rep2", tag="rep2")
        for h in range(2):
            rows = slice(64 * h, 64 * h + 64)
            nc.vector.scalar_tensor_tensor(
                rep2[rows, :], av_ps[rows, ds(64 * h, 64)], srz[rows, :],
                rep_q[rows, p, :], op0=ALU.mult, op1=ALU.add,
            )
        # rep2T block-diag (zeros kill cross-head terms in S2)
        r2t_ps = psum.tile([64, 128], BF16, name="r2t_ps", tag="b3s", bufs=1)
        nc.tensor.transpose(r2t_ps, rep2, ident_bf)
        r2bd = st2.tile([128, 128], BF16, name="r2bd", tag="r2bd")
        nc.vector.memset(r2bd, 0.0)
        for h in range(2):
            nc.vector.tensor_copy(
                r2bd[ds(64 * h, 64), ds(64 * h, 64)],
                r2t_ps[:, ds(64 * h, 64)],
            )
        # S2 = rep2 @ rep2^T (both heads at once); exp(SCALE*S2) on the
        # diagonal quadrants only (cross quadrants stay zero).  exp(S2) is
        # symmetric, so it serves directly as lhsT for xd2 = P2 @ rep2.
        s2_ps = psum.tile([128, 128], F32, name="s2_ps", tag="b3s", bufs=1)
        nc.tensor.matmul(s2_ps, r2bd, r2bd, start=True, stop=True)
        es = st2.tile([128, 128], BF16, name="es", tag="es")
        nc.vector.memset(es, 0.0)
        z2 = stats.tile([128, 1], F32, name="z2", tag="z2")
        for h in range(2):
            rows = slice(64 * h, 64 * h + 64)
            nc.scalar.activation(
                out=es[rows, ds(64 * h, 64)], in_=s2_ps[rows, ds(64 * h, 64)],
                func=ACTF.Exp, bias=0.0, scale=SCALE,
                accum_out=z2[rows, :],
            )
        rz2 = stats.tile([128, 1], F32, name="rz2", tag="rz2")
        nc.vector.reciprocal(rz2, z2)
        xd_ps = psum.tile([128, NB], F32, name="xd_ps", tag="b3s", bufs=1)
        nc.tensor.matmul(xd_ps, es, rep2, start=True, stop=True)
        # xd2_bd: block-diagonal stationary for the back-projection, with
        # rz (stage-1 softmax), rz2 (stage-2 softmax) and step_x folded in.
        rzsx = stats.tile([128, 1], F32, name="rzsx", tag="rzsx")
        nc.vector.tensor_mul(rzsx, rz, sx_bc[:, p : p + 1])
        nc.vector.tensor_mul(rzsx, rzsx, rz2)
        xd2_bd = xdtp.tile([128, 128], BF16, name="xd2_bd", tag=f"xd2_{p}",
                           bufs=1)
        nc.vector.memset(xd2_bd, 0.0)
        for h in range(2):
            rows = slice(64 * h, 64 * h + 64)
            nc.vector.tensor_scalar_mul(
                xd2_bd[rows, ds(64 * h, 64)], xd_ps[rows, :], rzsx[rows, :],
            )
        xd2_tiles.append(xd2_bd)

    def _bp_phase(p):
        # back-projection for one pair: the xd2 stationary is loaded once
        # and streams 8 chunk matmuls.  Pairs 0-2 run under the attention
        # pipeline; only pair 3's back-projection is left for the tail.
        for j2 in range(NCHUNK // 2):
            bp_ps = psum.tile(
                [128, 2, 512], F32, name="bp_ps", tag="mm", bufs=2)
            for ji in range(2):
                nc.tensor.matmul(
                    bp_ps[:, ji, :], xd2_tiles[p],
                    p_tiles[p][:, ts(2 * j2 + ji, 512)],
                    start=True, stop=True,
                )
            xdt_sb = xdtp.tile(
                [128, 2, 512], BF16, name="xdt_sb",
                tag=f"xdt_{p}_{j2}", bufs=1)
            if j2 % 2 == 0:
                nc.vector.tensor_copy(xdt_sb, bp_ps)
            else:
                nc.scalar.copy(xdt_sb, bp_ps)
            xdt_tiles[p].append(xdt_sb)

    # software pipeline with a 1-pair offset: the PE fills the wait for
    # pair p's P^T (exp + DMA transpose) with pair p-1's delta/stage-2
    # work; back-projections are emitted two steps after their pair's
    # scores so the in-order PE queue never waits on a stage-2 chain.
    for step in range(NPAIR + 2):
        if step < NPAIR:
            _score_phase(step)
        if 1 <= step <= NPAIR:
            _delta_phase(step - 1)
        if step >= 2:
            _bp_phase(step - 2)
    prp.release()
    st2.release()
    ptp.release()
    pp.release()

    # ---- output projection, per (chunk, 128-channel group) ---------------
    for j in range(NCHUNK):
        for ct in range(2):
            op_ps = psum.tile([128, 512], F32, name="op_ps", tag="outp",
                              bufs=2)
            for g in range(4):
                nc.tensor.matmul(
                    op_ps, wo_bf[:, g, ts(ct, 128)],
                    xdt_tiles[g][j // 2][:, j % 2, :],
                    start=(g == 0), stop=False,
                )
            nc.tensor.matmul(
                op_ps, bo_sb[:, ct, :], ones_row, start=False, stop=True,
            )
            out_sb = xdtp.tile([128, 512], F32, name="out_sb", tag="out_sb",
                               bufs=4)
            nc.scalar.copy(out_sb[:, 0:256], op_ps[:, 0:256])
            nc.vector.tensor_copy(out_sb[:, 256:512], op_ps[:, 256:512])
            nc.sync.dma_start(out_r[:, ct, ts(j, 512)], out_sb)
    xdtp.release()
    psum.release()
    stats.release()
    consts.release()


_CACHE = {}


class _Runner:
    """Builds the Bass module once and keeps a single jitted shard_map
    executable alive, so repeat kernel() calls skip retracing/relowering."""

    def __init__(self):
        import jax
        import jax.numpy as jnp
        from jax.sharding import Mesh, PartitionSpec
        from jax.experimental.shard_map import shard_map
        from concourse import bass2jax

        self.jax = jax
        nc = build_module()
        self.nc = nc
        bass2jax.install_neuronx_cc_hook()

        partition_name = (
            nc.partition_id_tensor.name if nc.partition_id_tensor else None
        )
        in_names, out_names, out_avals = [], [], []
        for alloc in nc.m.functions[0].allocations:
            if not isinstance(alloc, mybir.MemoryLocationSet):
                continue
            name = alloc.memorylocations[0].name
            if alloc.kind == "ExternalInput":
                if name != partition_name:
                    in_names.append(name)
            elif alloc.kind == "ExternalOutput":
                out_names.append(name)
                out_avals.append(
                    jax.core.ShapedArray(
                        tuple(alloc.tensor_shape), mybir.dt.np(alloc.dtype)
                    )
                )
        n_params = len(in_names)
        n_outs = len(out_avals)
        all_names = list(in_names) + list(out_names)
        if partition_name is not None:
            all_names.append(partition_name)
        self.in_names = in_names
        self.out_names = out_names
        self.out_avals = out_avals

        def _body(*args):
            operands = list(args)
            if partition_name is not None:
                operands.append(bass2jax.partition_id_tensor())
            outs = bass2jax._bass_exec_p.bind(
                *operands,
                out_avals=tuple(out_avals),
                in_names=tuple(all_names),
                out_names=tuple(out_names),
                lowering_input_output_aliases=(),
                sim_require_finite=True,
                sim_require_nnan=True,
                nc=nc,
            )
            return tuple(outs)

        self.body = _body
        devices = jax.devices()[:B]
        mesh = Mesh(np.asarray(devices), ("core",))
        donate = tuple(range(n_params, n_params + n_outs))
        self.sharded = jax.jit(
            shard_map(
                _body, mesh=mesh,
                in_specs=(PartitionSpec("core"),) * (n_params + n_outs),
                out_specs=(PartitionSpec("core"),) * n_outs,
                check_rep=False,
            ),
            donate_argnums=donate,
            keep_unused=True,
        )

    def run(self, in_maps):
        concat_in = [
            np.concatenate([m[name] for m in in_maps], axis=0)
            for name in self.in_names
        ]
        zeros = [
            np.zeros((B * a.shape[0], *a.shape[1:]), a.dtype) for a in self.out_avals
        ]
        out_arrs = self.sharded(*concat_in, *zeros)
        return [
            {
                name: np.asarray(out_arrs[i]).reshape(B, *self.out_avals[i].shape)[c]
                for i, name in enumerate(self.out_names)
            }
            for c in range(B)
        ]

    def bench(self, in_maps, reps=8, inner=72, base=8):
        """Time device-resident executions (no donation, operands staged once).

        Times jitted chains of `base` and `inner` back-to-back kernel
        executions; returns (per_exec_seconds, base_chain_seconds, results)
        with per_exec = (t_inner - t_base) / (inner - base), which amortizes
        away the per-dispatch round-trip of this axon-tunneled environment.
        """
        import time
        from jax.sharding import Mesh, PartitionSpec, NamedSharding
        from jax.experimental.shard_map import shard_map

        jax = self.jax
        devices = jax.devices()[:B]
        mesh = Mesh(np.asarray(devices), ("core",))
        sharding = NamedSharding(mesh, PartitionSpec("core"))
        n_ops = len(self.in_names) + len(self.out_avals)

        def chain(n):
            def f(*args):
                outs = []
                for _ in range(n):
                    outs.extend(self.body(*args))
                return tuple(outs)
            return f

        concat_in = [
            np.concatenate([m[name] for m in in_maps], axis=0)
            for name in self.in_names
        ]
        zeros = [
            np.zeros((B * a.shape[0], *a.shape[1:]), a.dtype) for a in self.out_avals
        ]
        staged = [jax.device_put(a, sharding) for a in concat_in + zeros]

        times = {}
        out1 = None
        for n in (base, inner):
            jfn = jax.jit(
                shard_map(
                    chain(n), mesh=mesh,
                    in_specs=(PartitionSpec("core"),) * n_ops,
                    out_specs=(PartitionSpec("core"),) * (n * len(self.out_avals)),
                    check_rep=False,
                ),
                keep_unused=True,
            )
            out = jfn(*staged)
            jax.block_until_ready(out)
            best = float("inf")
            for _ in range(reps):
                t0 = time.perf_counter()
                out = jfn(*staged)
                jax.block_until_ready(out)
                best = min(best, time.perf_counter() - t0)
            times[n] = best
            if n == base:
                out1 = out
        per_exec = (times[inner] - times[base]) / (inner - base)
        if per_exec <= 0:
            per_exec = times[inner] / inner  # noise floor: report upper bound
        results = [
            {
                name: np.asarray(out1[i]).reshape(B, *self.out_avals[i].shape)[c]
                for i, name in enumerate(self.out_names)
            }
            for c in range(B)
        ]
        return per_exec, times[base], results


def _get_runner():
    key = tuple(sorted(CFG.items()))
    if key not in _CACHE:
        _CACHE[key] = _Runner()
    return _CACHE[key]


def _pool_mask():
    import ml_dtypes

    mask = np.zeros((128, 8, NB), dtype=np.float32)
    for tp in range(128):
        wb = (tp % 64) // 8
        for br in range(8):
            mask[tp, br, 8 * br + wb] = 1.0 / 64.0
    return mask.astype(ml_dtypes.bfloat16)


def _make_in_maps(x, W_proj, step_rep, step_x, W_out, b_out):
    x = np.ascontiguousarray(np.asarray(x, dtype=np.float32))
    shared = {
        "w_proj": np.ascontiguousarray(np.asarray(W_proj, dtype=np.float32)),
        "w_out": np.ascontiguousarray(np.asarray(W_out, dtype=np.float32)),
        "b_out": np.ascontiguousarray(np.asarray(b_out, dtype=np.float32)),
        "s_rep": np.ascontiguousarray(
            np.asarray(step_rep, dtype=np.float32).reshape(HEADS)
        ),
        "s_x": np.ascontiguousarray(
            np.asarray(step_x, dtype=np.float32).reshape(HEADS)
        ),
        "pool_mask": _pool_mask(),
    }
    return [
        {"x": np.ascontiguousarray(x[b].reshape(C, T)), **shared} for b in range(B)
    ]


def kernel(x, W_proj, step_rep, step_x, W_out, b_out):
    runner = _get_runner()
    results = runner.run(_make_in_maps(x, W_proj, step_rep, step_x, W_out, b_out))
    outs = [np.asarray(results[b]["out"]).reshape(C, 64, 64) for b in range(B)]
    return np.stack(outs, axis=0)


# revision 57
# speedup vs baseline: 1.0399x; 1.0011x over previous
"""Trainium2 Bass kernel for nn_CBSA_45389214384209 (sparse_attention).

Reference computation (per batch element b of 8):
  x_seq = x[b].T                      # [4096, 256]   (x[b] is [256, 4096])
  proj  = x_seq @ W_proj              # [4096, 512]
  rep   = avgpool8x8(proj)            # [64, 512]  == avgpool8x8(x_seq) @ W_proj
  per head h (8 heads, dh=64):
    S    = rep_h @ proj_h.T * scale   # [64, 4096]
    P    = softmax(S)                 # [64, 4096]
    rd   = P @ proj_h                 # [64, 64]
    rep2 = rep_h + step_rep[h] * rd
    P2   = softmax(rep2 @ rep2.T * scale)
    xd2  = step_x[h] * (P2 @ rep2)    # [64, 64]
    xdT  = xd2.T @ P                  # [64, 4096]  (back-projection, transposed)
  out[b] = W_out.T @ concat_h(xdT) + b_out[:, None]   # [256, 4096]

Sharding: pure data parallel - one batch element per NeuronCore (8 cores).

Layout strategy: projections are computed "transposed" (feature dim on
partitions) straight from the native DRAM layout of x; the final
x_outT[c, t] is exactly the DRAM layout of the output.  Head pairs are
packed on partitions (rows 0-63 = even head, 64-127 = odd head) with
block-diagonal stationary tiles so one 128-wide matmul serves two heads.
P^T (needed by the rep_delta contraction over tokens) is produced by the
DMA xbar transpose engine -- the PE never transposes the big P tiles.
rep_delta is accumulated in [query, dim] orientation so every softmax
normalization is a per-partition scalar multiply.  exp(S2) is symmetric,
which lets stage 2 skip the P2 transpose.  The output bias is folded into
the last matmul as a K=1 rank-1 update and the result is DMA'd to DRAM
directly from PSUM.
"""

import os
import sys

import numpy as np

for _p in ("/opt/trn_rl_repo", os.path.expanduser("~/.axon_site/_ro/trn_rl_repo")):
    if os.path.isdir(_p) and _p not in sys.path:
        sys.path.insert(0, _p)

import concourse.bass as bass
import concourse.tile as tile
from concourse import bacc, mybir
from concourse.bass import ds, ts
from concourse.masks import make_identity

F32 = mybir.dt.float32
F32R = mybir.dt.float32r
BF16 = mybir.dt.bfloat16
AX = mybir.AxisListType
ALU = mybir.AluOpType
ACTF = mybir.ActivationFunctionType

B = 8
C = 256          # model dim
T = 4096         # tokens (64x64 grid)
INNER = 512
HEADS = 8
DH = 64
NB = 64          # pooled tokens (8x8 grid)
SCALE = DH ** -0.5
NPAIR = 4        # head pairs
NCHUNK = 8       # 512-wide token chunks
NTT = 32         # 128-wide token tiles

CFG = {
    "dmat": True,             # DMA-xbar transpose for P^T (else PE transposes)
    "bf16_pass": False,       # bf16 projection passes (slower: swdge casts)
}


def build_module(cfg=CFG):
    nc = bacc.Bacc("TRN2", debug=False)

    io_dt = F32 if cfg["bf16_pass"] else F32R
    x = nc.dram_tensor("x", [C, T], io_dt, kind="ExternalInput").ap()
    wp = nc.dram_tensor("w_proj", [C, INNER], io_dt, kind="ExternalInput").ap()
    wo = nc.dram_tensor("w_out", [INNER, C], F32, kind="ExternalInput").ap()
    bo = nc.dram_tensor("b_out", [C], F32R, kind="ExternalInput").ap()
    srep = nc.dram_tensor("s_rep", [HEADS], F32, kind="ExternalInput").ap()
    sx = nc.dram_tensor("s_x", [HEADS], F32, kind="ExternalInput").ap()
    pmask = nc.dram_tensor(
        "pool_mask", [128, 8, NB], BF16, kind="ExternalInput"
    ).ap()
    out = nc.dram_tensor("out", [C, T], F32, kind="ExternalOutput").ap()

    with tile.TileContext(nc) as tc:
        _body(tc, cfg, x, wp, wo, bo, srep, sx, pmask, out)
    nc.compile()
    return nc


def _body(tc, cfg, x, wp, wo, bo, srep, sx, pmask, out):
    nc = tc.nc

    x_r = x.rearrange("(o p) t -> p o t", p=128)      # [128, 2, 4096]
    out_r = out.rearrange("(o p) t -> p o t", p=128)  # [128, 2, 4096]

    # ---- pools (SBUF pools stack-nested: alloc order == reverse release) --
    consts = tc.alloc_tile_pool(name="consts", bufs=1)
    stats = tc.alloc_tile_pool(name="stats", bufs=2)
    xdtp = tc.alloc_tile_pool(name="xdtp", bufs=2)       # xd2_bd + xdT chunks
    pp = tc.alloc_tile_pool(name="pp", bufs=1)           # P (attn) tiles
    ptp = tc.alloc_tile_pool(name="ptp", bufs=1)         # P^T tiles
    st2 = tc.alloc_tile_pool(name="st2", bufs=2)         # stage-2 temps
    prp = tc.alloc_tile_pool(name="prp", bufs=1)         # proj (bf16)
    pTp = tc.alloc_tile_pool(name="pTp", bufs=1)         # projT (bf16)
    xp = tc.alloc_tile_pool(name="xp", bufs=1)           # x (f32r)

    # PSUM tags: mm [128,2,512] x2 (4 banks) + outp x1 (2) + acc x1 + b3s x1.
    psum = tc.alloc_tile_pool(name="psum", bufs=1, space="PSUM")

    # ---- constants / weights (direct f32r DMA: same bits as f32) ---------
    # DMA issue order on the sync queue matters: wp + x chunks first (they
    # gate pass 1); everything needed later is issued after the x stream.
    ident_bf = consts.tile([128, 128], BF16, name="ident_bf")
    make_identity(nc, ident_bf)
    ident_f = consts.tile([128, 128], F32, name="ident_f")
    make_identity(nc, ident_f)

    P_DT = BF16 if cfg["bf16_pass"] else F32R
    wp_r = consts.tile([128, 2, INNER], P_DT, name="wp_r")
    if cfg["bf16_pass"]:
        nc.gpsimd.dma_start(wp_r, wp.rearrange("(o p) i -> p o i", p=128))
    else:
        nc.sync.dma_start(wp_r, wp.rearrange("(o p) i -> p o i", p=128))
    pmask_sb = consts.tile([128, 8, NB], BF16, name="pmask_sb")
    nc.sync.dma_start(pmask_sb, pmask)

    # ---- fused pass over x: projT (bf16) + proj (bf16), per 512-chunk ----
    # x streams through a 2-deep window; both projection orientations are
    # produced from the same resident chunk.
    projT = pTp.tile([128, 4, T], BF16, name="projT")
    proj_bf = prp.tile([128, NTT, INNER], BF16, name="proj_bf")
    # rep[q, d] = pool_mask^T @ proj accumulates on the PE inside the same
    # loop: pool_mask[:, br, :] has 1/64 at column (8*br + wb) for block-row
    # br; each 128-token tile lies in exactly one block-row (br = m//4).
    rep_pst = psum.tile([128, 512], F32, name="rep_pst", tag="outp", bufs=2)
    rep_ps = rep_pst[0:64, :]
    for j in range(NCHUNK):
        x_c = xp.tile([128, 2, 512], P_DT, name="x_c", tag="xc", bufs=2)
        if cfg["bf16_pass"]:
            nc.gpsimd.dma_start(x_c, x_r[:, :, ts(j, 512)])
        else:
            nc.sync.dma_start(x_c, x_r[:, :, ts(j, 512)])
        for gp in range(2):
            pt_ps = psum.tile([128, 2, 512], F32, name="pt_ps", tag="mm", bufs=2)
            for gi in range(2):
                g = 2 * gp + gi
                for o in range(2):
                    nc.tensor.matmul(
                        pt_ps[:, gi, :], wp_r[:, o, ts(g, 128)], x_c[:, o, :],
                        start=(o == 0), stop=(o == 1),
                    )
            if gp == 0:
                nc.scalar.copy(projT[:, ds(0, 2), ts(j, 512)], pt_ps)
            else:
                nc.vector.tensor_copy(projT[:, ds(2, 2), ts(j, 512)], pt_ps)
        for mp in range(2):
            pr_ps = psum.tile([128, 2, 512], F32, name="pr_ps", tag="mm", bufs=2)
            for mi in range(2):
                k = 2 * mp + mi
                for o in range(2):
                    nc.tensor.matmul(
                        pr_ps[:, mi, :], x_c[:, o, ts(k, 128)], wp_r[:, o, :],
                        start=(o == 0), stop=(o == 1),
                    )
            if mp == 0:
                nc.vector.tensor_copy(proj_bf[:, ds(4 * j, 2), :], pr_ps)
            else:
                nc.scalar.copy(proj_bf[:, ds(4 * j + 2, 2), :], pr_ps)
        # pool-MMs run one chunk behind so they never wait on the proj
        # copies of the chunk the PE just produced
        for k in range(4):
            if j == 0:
                break
            m = 4 * (j - 1) + k
            nc.tensor.matmul(
                rep_ps, pmask_sb[:, m // 4, :], proj_bf[:, m, :],
                start=(m == 0), stop=False,
            )
    for k in range(4):
        m = 4 * (NCHUNK - 1) + k
        nc.tensor.matmul(
            rep_ps, pmask_sb[:, m // 4, :], proj_bf[:, m, :],
            start=False, stop=(m == NTT - 1),
        )
    xp.release()

    # late-needed constants, issued behind the x stream
    wo_bf = consts.tile([128, 4, C], BF16, name="wo_bf")
    nc.gpsimd.dma_start(wo_bf, wo.rearrange("(g p) c -> p g c", p=128))
    bo_sb = consts.tile([1, 2, C // 2], F32R, name="bo_sb")
    nc.sync.dma_start(
        bo_sb,
        bass.AP(tensor=bo.tensor, offset=bo.offset,
                ap=[[0, 1], [C // 2, 2], [1, C // 2]]),
    )
    pmask_sb = consts.tile([128, 8, NB], BF16, name="pmask_sb")
    nc.sync.dma_start(pmask_sb, pmask)
    ones_f = consts.tile([1, 512], F32, name="ones_f")
    nc.vector.memset(ones_f, 1.0)
    ones_row = consts.tile([1, 512], F32R, name="ones_row")
    nc.vector.tensor_copy(ones_row, ones_f)

    # step_rep / step_x: column p holds step[2p] on partitions 0-63 and
    # step[2p+1] on partitions 64-127 (per-q-partition scalars).
    srep_ld = consts.tile([128, HEADS], F32, name="srep_ld")
    sx_ld = consts.tile([128, HEADS], F32, name="sx_ld")
    srep_bc = consts.tile([128, NPAIR], F32, name="srep_bc")
    sx_bc = consts.tile([128, NPAIR], F32, name="sx_bc")
    for st_dram, st_ld, st_bc in ((srep, srep_ld, srep_bc), (sx, sx_ld, sx_bc)):
        bcast = bass.AP(
            tensor=st_dram.tensor, offset=st_dram.offset,
            ap=[[0, 128], [st_dram.ap[0][0], HEADS]],
        )
        nc.sync.dma_start(st_ld, bcast)
        st_ldv = st_ld.rearrange("p (c two) -> p c two", two=2)
        for half in range(2):
            rows = slice(64 * half, 64 * half + 64)
            nc.vector.tensor_copy(st_bc[rows, :], st_ldv[rows, :, half])

    # ---- rep staging: repT_bd + rep_q from the pooled rep ----------------
    rep_sb = consts.tile([64, INNER], F32, name="rep_sb")
    nc.scalar.copy(rep_sb, rep_ps)
    # move a copy of rep onto partitions 64-127 (for the qB half of rep_q)
    qb_ps = psum.tile([128, 512], F32, name="qb_ps", tag="outp", bufs=2)
    nc.tensor.matmul(
        qb_ps[64:128, :], ident_f[0:64, 0:64], rep_sb, start=True, stop=True,
    )
    # repT_bd: scaled block-diagonal [[sA, 0], [0, sB]] per pair (one K=128
    # matmul computes both heads' scores; zeros kill the cross terms), and
    # rep_q[p] = [128 (qA|qB), 64 d] for the stage-2 update.
    repT_bd = consts.tile([128, NPAIR, 128], BF16, name="repT_bd")
    nc.vector.memset(repT_bd, 0.0)
    rep_q = consts.tile([128, NPAIR, NB], F32, name="rep_q")
    for g in range(4):
        rT_ps = psum.tile([128, NB], F32, name="rT_ps", tag="acc", bufs=1)
        nc.tensor.transpose(
            rT_ps, rep_sb[:, ds(128 * g, 128)], ident_f[0:64, 0:64]
        )
        for h in range(2):
            rows = slice(64 * h, 64 * h + 64)
            nc.vector.tensor_scalar_mul(
                repT_bd[rows, g, ds(64 * h, 64)], rT_ps[rows, :], SCALE
            )
        nc.vector.tensor_copy(rep_q[0:64, g, :], rep_sb[:, ds(128 * g, 64)])
        nc.vector.tensor_copy(
            rep_q[64:128, g, :], qb_ps[64:128, ds(128 * g + 64, 64)]
        )

    # ---- attention stages, pipelined per head pair ----------------------
    # P_sb[p]: [128, T]; rows 0-63 = head 2p, rows 64-127 = head 2p+1.
    # Scores are q.k/8 with pooled queries -> bounded |s| ~< 2, so softmax
    # needs no max-subtraction; exp reads score pairs straight from PSUM.
    # P stays unnormalized; 1/Z folds into rep_delta (per-partition) and
    # into the back-projection stationary (rz*rz2*step_x on xd2).
    p_tiles = [
        pp.tile([128, T], BF16, name=f"p{p}", tag=f"p{p}") for p in range(NPAIR)
    ]
    xd2_tiles = []
    xdt_tiles = [[] for _ in range(NPAIR)]
    pt_list = [None] * NPAIR
    rz_list = [None] * NPAIR

    def _score_phase(p):
        # S + exp + P^T for pair p.  Sub-DMA-transposes chase the exp stream
        # so pt is ready ~one group after the last exp, not 3.6us later.
        zpart = stats.tile([128, 4], F32, name="zpart", tag="zpart")
        pt_t = ptp.tile([128, NTT, 128], BF16, name=f"pt{p}", tag="pt", bufs=4)
        for jp in range(4):
            s_ps = psum.tile([128, 2, 512], F32, name="s_ps", tag="mm", bufs=2)
            for ji in range(2):
                nc.tensor.matmul(
                    s_ps[:, ji, :], repT_bd[:, p, :],
                    projT[:, p, ts(2 * jp + ji, 512)],
                    start=True, stop=True,
                )
            nc.scalar.activation(
                out=p_tiles[p][:, ts(jp, 1024)].rearrange(
                    "p (a b) -> p a b", a=2),
                in_=s_ps, func=ACTF.Exp,
                bias=0.0, scale=1.0, accum_out=zpart[:, jp : jp + 1],
            )
            if cfg["dmat"]:
                nc.sync.dma_start_transpose(
                    pt_t[:, ds(8 * jp, 8), :],
                    p_tiles[p][:, ts(jp, 1024)],
                )
            else:
                for mq in range(2):
                    tp_ps = psum.tile(
                        [128, 4, 128], BF16, name="tp_ps", tag="b3s", bufs=1)
                    for c4 in range(4):
                        nc.tensor.transpose(
                            tp_ps[:, c4, :],
                            p_tiles[p][:, ts(8 * jp + 4 * mq + c4, 128)],
                            ident_bf)
                    nc.vector.tensor_copy(
                        pt_t[:, ds(8 * jp + 4 * mq, 4), :], tp_ps)
        zsum = stats.tile([128, 1], F32, name="zsum", tag="zsum")
        nc.vector.reduce_sum(zsum, zpart, axis=AX.X)
        rz = stats.tile([128, 1], F32, name="rz", tag=f"rzp{p}", bufs=1)
        nc.vector.reciprocal(rz, zsum)
        pt_list[p] = pt_t
        rz_list[p] = rz
        if p == NPAIR - 1:
            pTp.release()

    def _delta_phase(p):
        rz = rz_list[p]
        # rep_delta [q, d]: av[q, d'] = sum_t P[q,t] proj[t,d'] accumulated
        # over 32 token tiles; diagonal 64x64 blocks are the two heads'.
        av_ps = psum.tile([128, 128], F32, name="av_ps", tag="acc", bufs=1)
        for m in range(NTT):
            nc.tensor.matmul(
                av_ps, pt_list[p][:, m, :], proj_bf[:, m, ds(128 * p, 128)],
                start=(m == 0), stop=(m == NTT - 1),
            )
        # rep2 = rep_q + (step_rep * rz) * rep_delta   [128 q-pair, 64 d]
        srz = stats.tile([128, 1], F32, name="srz", tag="srz")
        nc.vector.tensor_mul(srz, rz, srep_bc[:, p : p + 1])
        rep2 = st2.tile([128, NB], BF16, name="rep2", tag="rep2")
        for h in range(2):
            rows = slice(64 * h, 64 * h + 64)
            nc.vector.scalar_tensor_tensor(
                rep2[rows, :], av_ps[rows, ds(64 * h, 64)], srz[rows, :],
                rep_q[rows, p, :], op0=ALU.mult, op1=ALU.add,
            )
        # rep2T block-diag (zeros kill cross-head terms in S2)
        r2t_ps = psum.tile([64, 128], BF16, name="r2t_ps", tag="b3s", bufs=1)
        nc.tensor.transpose(r2t_ps, rep2, ident_bf)
        r2bd = st2.tile([128, 128], BF16, name="r2bd", tag="r2bd")
        nc.vector.memset(r2bd, 0.0)
        for h in range(2):
            nc.vector.tensor_copy(
                r2bd[ds(64 * h, 64), ds(64 * h, 64)],
                r2t_ps[:, ds(64 * h, 64)],
            )
        # S2 = rep2 @ rep2^T (both heads at once); exp(SCALE*S2) on the
        # diagonal quadrants only (cross quadrants stay zero).  exp(S2) is
        # symmetric, so it serves directly as lhsT for xd2 = P2 @ rep2.
        s2_ps = psum.tile([128, 128], F32, name="s2_ps", tag="b3s", bufs=1)
        nc.tensor.matmul(s2_ps, r2bd, r2bd, start=True, stop=True)
        es = st2.tile([128, 128], BF16, name="es", tag="es")
        nc.vector.memset(es, 0.0)
        z2 = stats.tile([128, 1], F32, name="z2", tag="z2")
        for h in range(2):
            rows = slice(64 * h, 64 * h + 64)
            nc.scalar.activation(
                out=es[rows, ds(64 * h, 64)], in_=s2_ps[rows, ds(64 * h, 64)],
                func=ACTF.Exp, bias=0.0, scale=SCALE,
                accum_out=z2[rows, :],
            )
        rz2 = stats.tile([128, 1], F32, name="rz2", tag="rz2")
        nc.vector.reciprocal(rz2, z2)
        xd_ps = psum.tile([128, NB], F32, name="xd_ps", tag="b3s", bufs=1)
        nc.tensor.matmul(xd_ps, es, rep2, start=True, stop=True)
        # xd2_bd: block-diagonal stationary for the back-projection, with
        # rz (stage-1 softmax), rz2 (stage-2 softmax) and step_x folded in.
        rzsx = stats.tile([128, 1], F32, name="rzsx", tag="rzsx")
        nc.vector.tensor_mul(rzsx, rz, sx_bc[:, p : p + 1])
        nc.vector.tensor_mul(rzsx, rzsx, rz2)
        xd2_bd = xdtp.tile([128, 128], BF16, name="xd2_bd", tag=f"xd2_{p}",
                           bufs=1)
        nc.vector.memset(xd2_bd, 0.0)
        for h in range(2):
            rows = slice(64 * h, 64 * h + 64)
            nc.vector.tensor_scalar_mul(
                xd2_bd[rows, ds(64 * h, 64)], xd_ps[rows, :], rzsx[rows, :],
            )
        xd2_tiles.append(xd2_bd)

    def _bp_phase(p):
        # back-projection for one pair: the xd2 stationary is loaded once
        # and streams 8 chunk matmuls.
        for j2 in range(NCHUNK // 2):
            bp_ps = psum.tile(
                [128, 2, 512], F32, name="bp_ps", tag="mm", bufs=2)
            for ji in range(2):
                nc.tensor.matmul(
                    bp_ps[:, ji, :], xd2_tiles[p],
                    p_tiles[p][:, ts(2 * j2 + ji, 512)],
                    start=True, stop=True,
                )
            xdt_sb = xdtp.tile(
                [128, 2, 512], BF16, name="xdt_sb",
                tag=f"xdt_{p}_{j2}", bufs=1)
            if j2 % 2 == 0:
                nc.vector.tensor_copy(xdt_sb, bp_ps)
            else:
                nc.scalar.copy(xdt_sb, bp_ps)
            xdt_tiles[p].append(xdt_sb)

    # software pipeline with a 1-pair offset: the PE fills the wait for
    # pair p's P^T (exp + DMA transpose) with pair p-1's delta/stage-2
    # work.  Back-projections come AFTER the next pair's scores in the
    # in-order PE queue: a bp gated on a stage-2 chain must never delay
    # the score matmuls that feed the (serial) exp stream.
    for step in range(NPAIR + 2):
        if step < NPAIR:
            _score_phase(step)
        if 1 <= step <= NPAIR:
            _delta_phase(step - 1)
        if step in (3, 4):
            _bp_phase(step - 3)
        if step == NPAIR + 1:
            _bp_phase(NPAIR - 2)
            _bp_phase(NPAIR - 1)
    prp.release()
    st2.release()
    ptp.release()
    pp.release()

    # ---- output projection, per (chunk, 128-channel group) ---------------
    for j in range(NCHUNK):
        for ct in range(2):
            op_ps = psum.tile([128, 512], F32, name="op_ps", tag="outp",
                              bufs=2)
            for g in range(4):
                nc.tensor.matmul(
                    op_ps, wo_bf[:, g, ts(ct, 128)],
                    xdt_tiles[g][j // 2][:, j % 2, :],
                    start=(g == 0), stop=False,
                )
            nc.tensor.matmul(
                op_ps, bo_sb[:, ct, :], ones_row, start=False, stop=True,
            )
            out_sb = xdtp.tile([128, 512], F32, name="out_sb", tag="out_sb",
                               bufs=4)
            nc.scalar.copy(out_sb[:, 0:256], op_ps[:, 0:256])
            nc.vector.tensor_copy(out_sb[:, 256:512], op_ps[:, 256:512])
            nc.sync.dma_start(out_r[:, ct, ts(j, 512)], out_sb)
    xdtp.release()
    psum.release()
    stats.release()
    consts.release()


_CACHE = {}


class _Runner:
    """Builds the Bass module once and keeps a single jitted shard_map
    executable alive, so repeat kernel() calls skip retracing/relowering."""

    def __init__(self):
        import jax
        import jax.numpy as jnp
        from jax.sharding import Mesh, PartitionSpec
        from jax.experimental.shard_map import shard_map
        from concourse import bass2jax

        self.jax = jax
        nc = build_module()
        self.nc = nc
        bass2jax.install_neuronx_cc_hook()

        partition_name = (
            nc.partition_id_tensor.name if nc.partition_id_tensor else None
        )
        in_names, out_names, out_avals = [], [], []
        for alloc in nc.m.functions[0].allocations:
            if not isinstance(alloc, mybir.MemoryLocationSet):
                continue
            name = alloc.memorylocations[0].name
            if alloc.kind == "ExternalInput":
                if name != partition_name:
                    in_names.append(name)
            elif alloc.kind == "ExternalOutput":
                out_names.append(name)
                out_avals.append(
                    jax.core.ShapedArray(
                        tuple(alloc.tensor_shape), mybir.dt.np(alloc.dtype)
                    )
                )
        n_params = len(in_names)
        n_outs = len(out_avals)
        all_names = list(in_names) + list(out_names)
        if partition_name is not None:
            all_names.append(partition_name)
        self.in_names = in_names
        self.out_names = out_names
        self.out_avals = out_avals

        def _body(*args):
            operands = list(args)
            if partition_name is not None:
                operands.append(bass2jax.partition_id_tensor())
            outs = bass2jax._bass_exec_p.bind(
                *operands,
                out_avals=tuple(out_avals),
                in_names=tuple(all_names),
                out_names=tuple(out_names),
                lowering_input_output_aliases=(),
                sim_require_finite=True,
                sim_require_nnan=True,
                nc=nc,
            )
            return tuple(outs)

        self.body = _body
        devices = jax.devices()[:B]
        mesh = Mesh(np.asarray(devices), ("core",))
        donate = tuple(range(n_params, n_params + n_outs))
        self.sharded = jax.jit(
            shard_map(
                _body, mesh=mesh,
                in_specs=(PartitionSpec("core"),) * (n_params + n_outs),
                out_specs=(PartitionSpec("core"),) * n_outs,
                check_rep=False,
            ),
            donate_argnums=donate,
            keep_unused=True,
        )

    def run(self, in_maps):
        concat_in = [
            np.concatenate([m[name] for m in in_maps], axis=0)
            for name in self.in_names
        ]
        zeros = [
            np.zeros((B * a.shape[0], *a.shape[1:]), a.dtype) for a in self.out_avals
        ]
        out_arrs = self.sharded(*concat_in, *zeros)
        return [
            {
                name: np.asarray(out_arrs[i]).reshape(B, *self.out_avals[i].shape)[c]
                for i, name in enumerate(self.out_names)
            }
            for c in range(B)
        ]

    def bench(self, in_maps, reps=8, inner=72, base=8):
        """Time device-resident executions (no donation, operands staged once).

        Times jitted chains of `base` and `inner` back-to-back kernel
        executions; returns (per_exec_seconds, base_chain_seconds, results)
        with per_exec = (t_inner - t_base) / (inner - base), which amortizes
        away the per-dispatch round-trip of this axon-tunneled environment.
        """
        import time
        from jax.sharding import Mesh, PartitionSpec, NamedSharding
        from jax.experimental.shard_map import shard_map

        jax = self.jax
        devices = jax.devices()[:B]
        mesh = Mesh(np.asarray(devices), ("core",))
        sharding = NamedSharding(mesh, PartitionSpec("core"))
        n_ops = len(self.in_names) + len(self.out_avals)

        def chain(n):
            def f(*args):
                outs = []
                for _ in range(n):
                    outs.extend(self.body(*args))
                return tuple(outs)
            return f

        concat_in = [
            np.concatenate([m[name] for m in in_maps], axis=0)
            for name in self.in_names
        ]
        zeros = [
            np.zeros((B * a.shape[0], *a.shape[1:]), a.dtype) for a in self.out_avals
        ]
        staged = [jax.device_put(a, sharding) for a in concat_in + zeros]

        times = {}
        out1 = None
        for n in (base, inner):
            jfn = jax.jit(
                shard_map(
                    chain(n), mesh=mesh,
                    in_specs=(PartitionSpec("core"),) * n_ops,
                    out_specs=(PartitionSpec("core"),) * (n * len(self.out_avals)),
                    check_rep=False,
                ),
                keep_unused=True,
            )
            out = jfn(*staged)
            jax.block_until_ready(out)
            best = float("inf")
            for _ in range(reps):
                t0 = time.perf_counter()
                out = jfn(*staged)
                jax.block_until_ready(out)
                best = min(best, time.perf_counter() - t0)
            times[n] = best
            if n == base:
                out1 = out
        per_exec = (times[inner] - times[base]) / (inner - base)
        if per_exec <= 0:
            per_exec = times[inner] / inner  # noise floor: report upper bound
        results = [
            {
                name: np.asarray(out1[i]).reshape(B, *self.out_avals[i].shape)[c]
                for i, name in enumerate(self.out_names)
            }
            for c in range(B)
        ]
        return per_exec, times[base], results


def _get_runner():
    key = tuple(sorted(CFG.items()))
    if key not in _CACHE:
        _CACHE[key] = _Runner()
    return _CACHE[key]


def _pool_mask():
    import ml_dtypes

    mask = np.zeros((128, 8, NB), dtype=np.float32)
    for tp in range(128):
        wb = (tp % 64) // 8
        for br in range(8):
            mask[tp, br, 8 * br + wb] = 1.0 / 64.0
    return mask.astype(ml_dtypes.bfloat16)


def _make_in_maps(x, W_proj, step_rep, step_x, W_out, b_out):
    x = np.ascontiguousarray(np.asarray(x, dtype=np.float32))
    shared = {
        "w_proj": np.ascontiguousarray(np.asarray(W_proj, dtype=np.float32)),
        "w_out": np.ascontiguousarray(np.asarray(W_out, dtype=np.float32)),
        "b_out": np.ascontiguousarray(np.asarray(b_out, dtype=np.float32)),
        "s_rep": np.ascontiguousarray(
            np.asarray(step_rep, dtype=np.float32).reshape(HEADS)
        ),
        "s_x": np.ascontiguousarray(
            np.asarray(step_x, dtype=np.float32).reshape(HEADS)
        ),
        "pool_mask": _pool_mask(),
    }
    return [
        {"x": np.ascontiguousarray(x[b].reshape(C, T)), **shared} for b in range(B)
    ]


def kernel(x, W_proj, step_rep, step_x, W_out, b_out):
    runner = _get_runner()
    results = runner.run(_make_in_maps(x, W_proj, step_rep, step_x, W_out, b_out))
    outs = [np.asarray(results[b]["out"]).reshape(C, 64, 64) for b in range(B)]
    return np.stack(outs, axis=0)


# revision 58
# speedup vs baseline: 1.0585x; 1.0179x over previous
"""Trainium2 Bass kernel for nn_CBSA_45389214384209 (sparse_attention).

Reference computation (per batch element b of 8):
  x_seq = x[b].T                      # [4096, 256]   (x[b] is [256, 4096])
  proj  = x_seq @ W_proj              # [4096, 512]
  rep   = avgpool8x8(proj)            # [64, 512]  == avgpool8x8(x_seq) @ W_proj
  per head h (8 heads, dh=64):
    S    = rep_h @ proj_h.T * scale   # [64, 4096]
    P    = softmax(S)                 # [64, 4096]
    rd   = P @ proj_h                 # [64, 64]
    rep2 = rep_h + step_rep[h] * rd
    P2   = softmax(rep2 @ rep2.T * scale)
    xd2  = step_x[h] * (P2 @ rep2)    # [64, 64]
    xdT  = xd2.T @ P                  # [64, 4096]  (back-projection, transposed)
  out[b] = W_out.T @ concat_h(xdT) + b_out[:, None]   # [256, 4096]

Sharding: pure data parallel - one batch element per NeuronCore (8 cores).

Layout strategy: projections are computed "transposed" (feature dim on
partitions) straight from the native DRAM layout of x; the final
x_outT[c, t] is exactly the DRAM layout of the output.  Head pairs are
packed on partitions (rows 0-63 = even head, 64-127 = odd head) with
block-diagonal stationary tiles so one 128-wide matmul serves two heads.
P^T (needed by the rep_delta contraction over tokens) is produced by the
DMA xbar transpose engine -- the PE never transposes the big P tiles.
rep_delta is accumulated in [query, dim] orientation so every softmax
normalization is a per-partition scalar multiply.  exp(S2) is symmetric,
which lets stage 2 skip the P2 transpose.  The output bias is folded into
the last matmul as a K=1 rank-1 update and the result is DMA'd to DRAM
directly from PSUM.
"""

import os
import sys

import numpy as np

for _p in ("/opt/trn_rl_repo", os.path.expanduser("~/.axon_site/_ro/trn_rl_repo")):
    if os.path.isdir(_p) and _p not in sys.path:
        sys.path.insert(0, _p)

import concourse.bass as bass
import concourse.tile as tile
from concourse import bacc, mybir
from concourse.bass import ds, ts
from concourse.masks import make_identity

F32 = mybir.dt.float32
F32R = mybir.dt.float32r
BF16 = mybir.dt.bfloat16
AX = mybir.AxisListType
ALU = mybir.AluOpType
ACTF = mybir.ActivationFunctionType

B = 8
C = 256          # model dim
T = 4096         # tokens (64x64 grid)
INNER = 512
HEADS = 8
DH = 64
NB = 64          # pooled tokens (8x8 grid)
SCALE = DH ** -0.5
NPAIR = 4        # head pairs
NCHUNK = 8       # 512-wide token chunks
NTT = 32         # 128-wide token tiles

CFG = {
    "dmat": True,             # DMA-xbar transpose for P^T (else PE transposes)
    "bf16_pass": False,       # bf16 projection passes (slower: swdge casts)
}


def build_module(cfg=CFG):
    nc = bacc.Bacc("TRN2", debug=False)

    io_dt = F32 if cfg["bf16_pass"] else F32R
    x = nc.dram_tensor("x", [C, T], io_dt, kind="ExternalInput").ap()
    wp = nc.dram_tensor("w_proj", [C, INNER], io_dt, kind="ExternalInput").ap()
    wo = nc.dram_tensor("w_out", [INNER, C], F32, kind="ExternalInput").ap()
    bo = nc.dram_tensor("b_out", [C], F32R, kind="ExternalInput").ap()
    srep = nc.dram_tensor("s_rep", [HEADS], F32, kind="ExternalInput").ap()
    sx = nc.dram_tensor("s_x", [HEADS], F32, kind="ExternalInput").ap()
    pmask = nc.dram_tensor(
        "pool_mask", [128, 8, NB], BF16, kind="ExternalInput"
    ).ap()
    out = nc.dram_tensor("out", [C, T], F32, kind="ExternalOutput").ap()

    with tile.TileContext(nc) as tc:
        _body(tc, cfg, x, wp, wo, bo, srep, sx, pmask, out)
    nc.compile()
    return nc


def _body(tc, cfg, x, wp, wo, bo, srep, sx, pmask, out):
    nc = tc.nc

    x_r = x.rearrange("(o p) t -> p o t", p=128)      # [128, 2, 4096]
    out_r = out.rearrange("(o p) t -> p o t", p=128)  # [128, 2, 4096]

    # ---- pools (SBUF pools stack-nested: alloc order == reverse release) --
    consts = tc.alloc_tile_pool(name="consts", bufs=1)
    stats = tc.alloc_tile_pool(name="stats", bufs=2)
    xdtp = tc.alloc_tile_pool(name="xdtp", bufs=2)       # xd2_bd + xdT chunks
    pp = tc.alloc_tile_pool(name="pp", bufs=1)           # P (attn) tiles
    ptp = tc.alloc_tile_pool(name="ptp", bufs=1)         # P^T tiles
    st2 = tc.alloc_tile_pool(name="st2", bufs=2)         # stage-2 temps
    prp = tc.alloc_tile_pool(name="prp", bufs=1)         # proj (bf16)
    pTp = tc.alloc_tile_pool(name="pTp", bufs=1)         # projT (bf16)
    xp = tc.alloc_tile_pool(name="xp", bufs=1)           # x (f32r)

    # PSUM tags: mm [128,2,512] x2 (4 banks) + outp x1 (2) + acc x1 + b3s x1.
    psum = tc.alloc_tile_pool(name="psum", bufs=1, space="PSUM")

    # ---- constants / weights (direct f32r DMA: same bits as f32) ---------
    # DMA issue order on the sync queue matters: wp + x chunks first (they
    # gate pass 1); everything needed later is issued after the x stream.
    ident_bf = consts.tile([128, 128], BF16, name="ident_bf")
    make_identity(nc, ident_bf)
    ident_f = consts.tile([128, 128], F32, name="ident_f")
    make_identity(nc, ident_f)

    P_DT = BF16 if cfg["bf16_pass"] else F32R
    wp_r = consts.tile([128, 2, INNER], P_DT, name="wp_r")
    if cfg["bf16_pass"]:
        nc.gpsimd.dma_start(wp_r, wp.rearrange("(o p) i -> p o i", p=128))
    else:
        nc.sync.dma_start(wp_r, wp.rearrange("(o p) i -> p o i", p=128))
    pmask_sb = consts.tile([128, 8, NB], BF16, name="pmask_sb")
    nc.sync.dma_start(pmask_sb, pmask)

    # ---- fused pass over x: projT (bf16) + proj (bf16), per 512-chunk ----
    # x streams through a 2-deep window; both projection orientations are
    # produced from the same resident chunk.
    projT = pTp.tile([128, 4, T], BF16, name="projT")
    proj_bf = prp.tile([128, NTT, INNER], BF16, name="proj_bf")
    # rep[q, d] = pool_mask^T @ proj accumulates on the PE inside the same
    # loop: pool_mask[:, br, :] has 1/64 at column (8*br + wb) for block-row
    # br; each 128-token tile lies in exactly one block-row (br = m//4).
    rep_pst = psum.tile([128, 512], F32, name="rep_pst", tag="outp", bufs=2)
    rep_ps = rep_pst[0:64, :]
    for j in range(NCHUNK):
        x_c = xp.tile([128, 2, 512], P_DT, name="x_c", tag="xc", bufs=2)
        if cfg["bf16_pass"]:
            nc.gpsimd.dma_start(x_c, x_r[:, :, ts(j, 512)])
        else:
            nc.sync.dma_start(x_c, x_r[:, :, ts(j, 512)])
        for gp in range(2):
            pt_ps = psum.tile([128, 2, 512], F32, name="pt_ps", tag="mm", bufs=2)
            for gi in range(2):
                g = 2 * gp + gi
                for o in range(2):
                    nc.tensor.matmul(
                        pt_ps[:, gi, :], wp_r[:, o, ts(g, 128)], x_c[:, o, :],
                        start=(o == 0), stop=(o == 1),
                    )
            if gp == 0:
                nc.scalar.copy(projT[:, ds(0, 2), ts(j, 512)], pt_ps)
            else:
                nc.vector.tensor_copy(projT[:, ds(2, 2), ts(j, 512)], pt_ps)
        for mp in range(2):
            pr_ps = psum.tile([128, 2, 512], F32, name="pr_ps", tag="mm", bufs=2)
            for mi in range(2):
                k = 2 * mp + mi
                for o in range(2):
                    nc.tensor.matmul(
                        pr_ps[:, mi, :], x_c[:, o, ts(k, 128)], wp_r[:, o, :],
                        start=(o == 0), stop=(o == 1),
                    )
            if mp == 0:
                nc.vector.tensor_copy(proj_bf[:, ds(4 * j, 2), :], pr_ps)
            else:
                nc.scalar.copy(proj_bf[:, ds(4 * j + 2, 2), :], pr_ps)
        # pool-MMs run one chunk behind so they never wait on the proj
        # copies of the chunk the PE just produced
        for k in range(4):
            if j == 0:
                break
            m = 4 * (j - 1) + k
            nc.tensor.matmul(
                rep_ps, pmask_sb[:, m // 4, :], proj_bf[:, m, :],
                start=(m == 0), stop=False,
            )
    for k in range(4):
        m = 4 * (NCHUNK - 1) + k
        nc.tensor.matmul(
            rep_ps, pmask_sb[:, m // 4, :], proj_bf[:, m, :],
            start=False, stop=(m == NTT - 1),
        )
    xp.release()

    # late-needed constants, issued behind the x stream
    wo_ld = consts.tile([128, 4, C], F32, name="wo_ld")
    nc.sync.dma_start(wo_ld, wo.rearrange("(g p) c -> p g c", p=128))
    wo_bf = consts.tile([128, 4, C], BF16, name="wo_bf")
    nc.vector.tensor_copy(wo_bf, wo_ld)
    bo_sb = consts.tile([1, 2, C // 2], F32R, name="bo_sb")
    nc.sync.dma_start(
        bo_sb,
        bass.AP(tensor=bo.tensor, offset=bo.offset,
                ap=[[0, 1], [C // 2, 2], [1, C // 2]]),
    )
    pmask_sb = consts.tile([128, 8, NB], BF16, name="pmask_sb")
    nc.sync.dma_start(pmask_sb, pmask)
    ones_f = consts.tile([1, 512], F32, name="ones_f")
    nc.vector.memset(ones_f, 1.0)
    ones_row = consts.tile([1, 512], F32R, name="ones_row")
    nc.vector.tensor_copy(ones_row, ones_f)

    # step_rep / step_x: column p holds step[2p] on partitions 0-63 and
    # step[2p+1] on partitions 64-127 (per-q-partition scalars).
    srep_ld = consts.tile([128, HEADS], F32, name="srep_ld")
    sx_ld = consts.tile([128, HEADS], F32, name="sx_ld")
    srep_bc = consts.tile([128, NPAIR], F32, name="srep_bc")
    sx_bc = consts.tile([128, NPAIR], F32, name="sx_bc")
    for st_dram, st_ld, st_bc in ((srep, srep_ld, srep_bc), (sx, sx_ld, sx_bc)):
        bcast = bass.AP(
            tensor=st_dram.tensor, offset=st_dram.offset,
            ap=[[0, 128], [st_dram.ap[0][0], HEADS]],
        )
        nc.sync.dma_start(st_ld, bcast)
        st_ldv = st_ld.rearrange("p (c two) -> p c two", two=2)
        for half in range(2):
            rows = slice(64 * half, 64 * half + 64)
            nc.vector.tensor_copy(st_bc[rows, :], st_ldv[rows, :, half])

    # ---- rep staging: repT_bd + rep_q from the pooled rep ----------------
    rep_sb = consts.tile([64, INNER], F32, name="rep_sb")
    nc.scalar.copy(rep_sb, rep_ps)
    # move a copy of rep onto partitions 64-127 (for the qB half of rep_q)
    qb_ps = psum.tile([128, 512], F32, name="qb_ps", tag="outp", bufs=2)
    nc.tensor.matmul(
        qb_ps[64:128, :], ident_f[0:64, 0:64], rep_sb, start=True, stop=True,
    )
    # repT_bd: scaled block-diagonal [[sA, 0], [0, sB]] per pair (one K=128
    # matmul computes both heads' scores; zeros kill the cross terms), and
    # rep_q[p] = [128 (qA|qB), 64 d] for the stage-2 update.
    repT_bd = consts.tile([128, NPAIR, 128], BF16, name="repT_bd")
    nc.vector.memset(repT_bd, 0.0)
    rep_q = consts.tile([128, NPAIR, NB], F32, name="rep_q")
    for g in range(4):
        rT_ps = psum.tile([128, NB], F32, name="rT_ps", tag="acc", bufs=1)
        nc.tensor.transpose(
            rT_ps, rep_sb[:, ds(128 * g, 128)], ident_f[0:64, 0:64]
        )
        for h in range(2):
            rows = slice(64 * h, 64 * h + 64)
            nc.vector.tensor_scalar_mul(
                repT_bd[rows, g, ds(64 * h, 64)], rT_ps[rows, :], SCALE
            )
        nc.vector.tensor_copy(rep_q[0:64, g, :], rep_sb[:, ds(128 * g, 64)])
        nc.vector.tensor_copy(
            rep_q[64:128, g, :], qb_ps[64:128, ds(128 * g + 64, 64)]
        )

    # ---- attention stages, pipelined per head pair ----------------------
    # P_sb[p]: [128, T]; rows 0-63 = head 2p, rows 64-127 = head 2p+1.
    # Scores are q.k/8 with pooled queries -> bounded |s| ~< 2, so softmax
    # needs no max-subtraction; exp reads score pairs straight from PSUM.
    # P stays unnormalized; 1/Z folds into rep_delta (per-partition) and
    # into the back-projection stationary (rz*rz2*step_x on xd2).
    p_tiles = [
        pp.tile([128, T], BF16, name=f"p{p}", tag=f"p{p}") for p in range(NPAIR)
    ]
    xd2_tiles = []
    xdt_tiles = [[] for _ in range(NPAIR)]
    pt_list = [None] * NPAIR
    rz_list = [None] * NPAIR

    def _score_phase(p):
        # S + exp + P^T for pair p.  Sub-DMA-transposes chase the exp stream
        # so pt is ready ~one group after the last exp, not 3.6us later.
        zpart = stats.tile([128, 4], F32, name="zpart", tag="zpart")
        pt_t = ptp.tile([128, NTT, 128], BF16, name=f"pt{p}", tag="pt", bufs=4)
        for jp in range(4):
            s_ps = psum.tile([128, 2, 512], F32, name="s_ps", tag="mm", bufs=2)
            for ji in range(2):
                nc.tensor.matmul(
                    s_ps[:, ji, :], repT_bd[:, p, :],
                    projT[:, p, ts(2 * jp + ji, 512)],
                    start=True, stop=True,
                )
            nc.scalar.activation(
                out=p_tiles[p][:, ts(jp, 1024)].rearrange(
                    "p (a b) -> p a b", a=2),
                in_=s_ps, func=ACTF.Exp,
                bias=0.0, scale=1.0, accum_out=zpart[:, jp : jp + 1],
            )
            if cfg["dmat"]:
                nc.sync.dma_start_transpose(
                    pt_t[:, ds(8 * jp, 8), :],
                    p_tiles[p][:, ts(jp, 1024)],
                )
            else:
                for mq in range(2):
                    tp_ps = psum.tile(
                        [128, 4, 128], BF16, name="tp_ps", tag="b3s", bufs=1)
                    for c4 in range(4):
                        nc.tensor.transpose(
                            tp_ps[:, c4, :],
                            p_tiles[p][:, ts(8 * jp + 4 * mq + c4, 128)],
                            ident_bf)
                    nc.vector.tensor_copy(
                        pt_t[:, ds(8 * jp + 4 * mq, 4), :], tp_ps)
        zsum = stats.tile([128, 1], F32, name="zsum", tag="zsum")
        nc.vector.reduce_sum(zsum, zpart, axis=AX.X)
        rz = stats.tile([128, 1], F32, name="rz", tag=f"rzp{p}", bufs=1)
        nc.vector.reciprocal(rz, zsum)
        pt_list[p] = pt_t
        rz_list[p] = rz
        if p == NPAIR - 1:
            pTp.release()

    def _delta_phase(p):
        rz = rz_list[p]
        # rep_delta [q, d]: av[q, d'] = sum_t P[q,t] proj[t,d'] accumulated
        # over 32 token tiles; diagonal 64x64 blocks are the two heads'.
        av_ps = psum.tile([128, 128], F32, name="av_ps", tag="acc", bufs=1)
        for m in range(NTT):
            nc.tensor.matmul(
                av_ps, pt_list[p][:, m, :], proj_bf[:, m, ds(128 * p, 128)],
                start=(m == 0), stop=(m == NTT - 1),
            )
        # rep2 = rep_q + (step_rep * rz) * rep_delta   [128 q-pair, 64 d]
        srz = stats.tile([128, 1], F32, name="srz", tag="srz")
        nc.vector.tensor_mul(srz, rz, srep_bc[:, p : p + 1])
        rep2 = st2.tile([128, NB], BF16, name="rep2", tag="rep2")
        for h in range(2):
            rows = slice(64 * h, 64 * h + 64)
            nc.vector.scalar_tensor_tensor(
                rep2[rows, :], av_ps[rows, ds(64 * h, 64)], srz[rows, :],
                rep_q[rows, p, :], op0=ALU.mult, op1=ALU.add,
            )
        # rep2T block-diag (zeros kill cross-head terms in S2)
        r2t_ps = psum.tile([64, 128], BF16, name="r2t_ps", tag="b3s", bufs=1)
        nc.tensor.transpose(r2t_ps, rep2, ident_bf)
        r2bd = st2.tile([128, 128], BF16, name="r2bd", tag="r2bd")
        nc.vector.memset(r2bd, 0.0)
        for h in range(2):
            nc.vector.tensor_copy(
                r2bd[ds(64 * h, 64), ds(64 * h, 64)],
                r2t_ps[:, ds(64 * h, 64)],
            )
        # S2 = rep2 @ rep2^T (both heads at once); exp(SCALE*S2) on the
        # diagonal quadrants only (cross quadrants stay zero).  exp(S2) is
        # symmetric, so it serves directly as lhsT for xd2 = P2 @ rep2.
        s2_ps = psum.tile([128, 128], F32, name="s2_ps", tag="b3s", bufs=1)
        nc.tensor.matmul(s2_ps, r2bd, r2bd, start=True, stop=True)
        es = st2.tile([128, 128], BF16, name="es", tag="es")
        nc.vector.memset(es, 0.0)
        z2 = stats.tile([128, 1], F32, name="z2", tag="z2")
        for h in range(2):
            rows = slice(64 * h, 64 * h + 64)
            nc.scalar.activation(
                out=es[rows, ds(64 * h, 64)], in_=s2_ps[rows, ds(64 * h, 64)],
                func=ACTF.Exp, bias=0.0, scale=SCALE,
                accum_out=z2[rows, :],
            )
        rz2 = stats.tile([128, 1], F32, name="rz2", tag="rz2")
        nc.vector.reciprocal(rz2, z2)
        xd_ps = psum.tile([128, NB], F32, name="xd_ps", tag="b3s", bufs=1)
        nc.tensor.matmul(xd_ps, es, rep2, start=True, stop=True)
        # xd2_bd: block-diagonal stationary for the back-projection, with
        # rz (stage-1 softmax), rz2 (stage-2 softmax) and step_x folded in.
        rzsx = stats.tile([128, 1], F32, name="rzsx", tag="rzsx")
        nc.vector.tensor_mul(rzsx, rz, sx_bc[:, p : p + 1])
        nc.vector.tensor_mul(rzsx, rzsx, rz2)
        xd2_bd = xdtp.tile([128, 128], BF16, name="xd2_bd", tag=f"xd2_{p}",
                           bufs=1)
        nc.vector.memset(xd2_bd, 0.0)
        for h in range(2):
            rows = slice(64 * h, 64 * h + 64)
            nc.vector.tensor_scalar_mul(
                xd2_bd[rows, ds(64 * h, 64)], xd_ps[rows, :], rzsx[rows, :],
            )
        xd2_tiles.append(xd2_bd)

    def _bp_phase(p):
        # back-projection for one pair: the xd2 stationary is loaded once
        # and streams 8 chunk matmuls.
        for j2 in range(NCHUNK // 2):
            bp_ps = psum.tile(
                [128, 2, 512], F32, name="bp_ps", tag="mm", bufs=2)
            for ji in range(2):
                nc.tensor.matmul(
                    bp_ps[:, ji, :], xd2_tiles[p],
                    p_tiles[p][:, ts(2 * j2 + ji, 512)],
                    start=True, stop=True,
                )
            xdt_sb = xdtp.tile(
                [128, 2, 512], BF16, name="xdt_sb",
                tag=f"xdt_{p}_{j2}", bufs=1)
            if j2 % 2 == 0:
                nc.vector.tensor_copy(xdt_sb, bp_ps)
            else:
                nc.scalar.copy(xdt_sb, bp_ps)
            xdt_tiles[p].append(xdt_sb)

    # software pipeline with a 1-pair offset: the PE fills the wait for
    # pair p's P^T (exp + DMA transpose) with pair p-1's delta/stage-2
    # work.  Back-projections come AFTER the next pair's scores in the
    # in-order PE queue: a bp gated on a stage-2 chain must never delay
    # the score matmuls that feed the (serial) exp stream.
    for step in range(NPAIR + 2):
        if step < NPAIR:
            _score_phase(step)
        if 1 <= step <= NPAIR:
            _delta_phase(step - 1)
        if step in (3, 4):
            _bp_phase(step - 3)
        if step == NPAIR + 1:
            _bp_phase(NPAIR - 2)
            _bp_phase(NPAIR - 1)
    prp.release()
    st2.release()
    ptp.release()
    pp.release()

    # ---- output projection, per (chunk, 128-channel group) ---------------
    for j in range(NCHUNK):
        for ct in range(2):
            op_ps = psum.tile([128, 512], F32, name="op_ps", tag="outp",
                              bufs=2)
            for g in range(4):
                nc.tensor.matmul(
                    op_ps, wo_bf[:, g, ts(ct, 128)],
                    xdt_tiles[g][j // 2][:, j % 2, :],
                    start=(g == 0), stop=False,
                )
            nc.tensor.matmul(
                op_ps, bo_sb[:, ct, :], ones_row, start=False, stop=True,
            )
            out_sb = xdtp.tile([128, 512], F32, name="out_sb", tag="out_sb",
                               bufs=4)
            nc.scalar.copy(out_sb[:, 0:256], op_ps[:, 0:256])
            nc.vector.tensor_copy(out_sb[:, 256:512], op_ps[:, 256:512])
            nc.sync.dma_start(out_r[:, ct, ts(j, 512)], out_sb)
    xdtp.release()
    psum.release()
    stats.release()
    consts.release()


_CACHE = {}


class _Runner:
    """Builds the Bass module once and keeps a single jitted shard_map
    executable alive, so repeat kernel() calls skip retracing/relowering."""

    def __init__(self):
        import jax
        import jax.numpy as jnp
        from jax.sharding import Mesh, PartitionSpec
        from jax.experimental.shard_map import shard_map
        from concourse import bass2jax

        self.jax = jax
        nc = build_module()
        self.nc = nc
        bass2jax.install_neuronx_cc_hook()

        partition_name = (
            nc.partition_id_tensor.name if nc.partition_id_tensor else None
        )
        in_names, out_names, out_avals = [], [], []
        for alloc in nc.m.functions[0].allocations:
            if not isinstance(alloc, mybir.MemoryLocationSet):
                continue
            name = alloc.memorylocations[0].name
            if alloc.kind == "ExternalInput":
                if name != partition_name:
                    in_names.append(name)
            elif alloc.kind == "ExternalOutput":
                out_names.append(name)
                out_avals.append(
                    jax.core.ShapedArray(
                        tuple(alloc.tensor_shape), mybir.dt.np(alloc.dtype)
                    )
                )
        n_params = len(in_names)
        n_outs = len(out_avals)
        all_names = list(in_names) + list(out_names)
        if partition_name is not None:
            all_names.append(partition_name)
        self.in_names = in_names
        self.out_names = out_names
        self.out_avals = out_avals

        def _body(*args):
            operands = list(args)
            if partition_name is not None:
                operands.append(bass2jax.partition_id_tensor())
            outs = bass2jax._bass_exec_p.bind(
                *operands,
                out_avals=tuple(out_avals),
                in_names=tuple(all_names),
                out_names=tuple(out_names),
                lowering_input_output_aliases=(),
                sim_require_finite=True,
                sim_require_nnan=True,
                nc=nc,
            )
            return tuple(outs)

        self.body = _body
        devices = jax.devices()[:B]
        mesh = Mesh(np.asarray(devices), ("core",))
        donate = tuple(range(n_params, n_params + n_outs))
        self.sharded = jax.jit(
            shard_map(
                _body, mesh=mesh,
                in_specs=(PartitionSpec("core"),) * (n_params + n_outs),
                out_specs=(PartitionSpec("core"),) * n_outs,
                check_rep=False,
            ),
            donate_argnums=donate,
            keep_unused=True,
        )

    def run(self, in_maps):
        concat_in = [
            np.concatenate([m[name] for m in in_maps], axis=0)
            for name in self.in_names
        ]
        zeros = [
            np.zeros((B * a.shape[0], *a.shape[1:]), a.dtype) for a in self.out_avals
        ]
        out_arrs = self.sharded(*concat_in, *zeros)
        return [
            {
                name: np.asarray(out_arrs[i]).reshape(B, *self.out_avals[i].shape)[c]
                for i, name in enumerate(self.out_names)
            }
            for c in range(B)
        ]

    def bench(self, in_maps, reps=8, inner=72, base=8):
        """Time device-resident executions (no donation, operands staged once).

        Times jitted chains of `base` and `inner` back-to-back kernel
        executions; returns (per_exec_seconds, base_chain_seconds, results)
        with per_exec = (t_inner - t_base) / (inner - base), which amortizes
        away the per-dispatch round-trip of this axon-tunneled environment.
        """
        import time
        from jax.sharding import Mesh, PartitionSpec, NamedSharding
        from jax.experimental.shard_map import shard_map

        jax = self.jax
        devices = jax.devices()[:B]
        mesh = Mesh(np.asarray(devices), ("core",))
        sharding = NamedSharding(mesh, PartitionSpec("core"))
        n_ops = len(self.in_names) + len(self.out_avals)

        def chain(n):
            def f(*args):
                outs = []
                for _ in range(n):
                    outs.extend(self.body(*args))
                return tuple(outs)
            return f

        concat_in = [
            np.concatenate([m[name] for m in in_maps], axis=0)
            for name in self.in_names
        ]
        zeros = [
            np.zeros((B * a.shape[0], *a.shape[1:]), a.dtype) for a in self.out_avals
        ]
        staged = [jax.device_put(a, sharding) for a in concat_in + zeros]

        times = {}
        out1 = None
        for n in (base, inner):
            jfn = jax.jit(
                shard_map(
                    chain(n), mesh=mesh,
                    in_specs=(PartitionSpec("core"),) * n_ops,
                    out_specs=(PartitionSpec("core"),) * (n * len(self.out_avals)),
                    check_rep=False,
                ),
                keep_unused=True,
            )
            out = jfn(*staged)
            jax.block_until_ready(out)
            best = float("inf")
            for _ in range(reps):
                t0 = time.perf_counter()
                out = jfn(*staged)
                jax.block_until_ready(out)
                best = min(best, time.perf_counter() - t0)
            times[n] = best
            if n == base:
                out1 = out
        per_exec = (times[inner] - times[base]) / (inner - base)
        if per_exec <= 0:
            per_exec = times[inner] / inner  # noise floor: report upper bound
        results = [
            {
                name: np.asarray(out1[i]).reshape(B, *self.out_avals[i].shape)[c]
                for i, name in enumerate(self.out_names)
            }
            for c in range(B)
        ]
        return per_exec, times[base], results


def _get_runner():
    key = tuple(sorted(CFG.items()))
    if key not in _CACHE:
        _CACHE[key] = _Runner()
    return _CACHE[key]


def _pool_mask():
    import ml_dtypes

    mask = np.zeros((128, 8, NB), dtype=np.float32)
    for tp in range(128):
        wb = (tp % 64) // 8
        for br in range(8):
            mask[tp, br, 8 * br + wb] = 1.0 / 64.0
    return mask.astype(ml_dtypes.bfloat16)


def _make_in_maps(x, W_proj, step_rep, step_x, W_out, b_out):
    x = np.ascontiguousarray(np.asarray(x, dtype=np.float32))
    shared = {
        "w_proj": np.ascontiguousarray(np.asarray(W_proj, dtype=np.float32)),
        "w_out": np.ascontiguousarray(np.asarray(W_out, dtype=np.float32)),
        "b_out": np.ascontiguousarray(np.asarray(b_out, dtype=np.float32)),
        "s_rep": np.ascontiguousarray(
            np.asarray(step_rep, dtype=np.float32).reshape(HEADS)
        ),
        "s_x": np.ascontiguousarray(
            np.asarray(step_x, dtype=np.float32).reshape(HEADS)
        ),
        "pool_mask": _pool_mask(),
    }
    return [
        {"x": np.ascontiguousarray(x[b].reshape(C, T)), **shared} for b in range(B)
    ]


def kernel(x, W_proj, step_rep, step_x, W_out, b_out):
    runner = _get_runner()
    results = runner.run(_make_in_maps(x, W_proj, step_rep, step_x, W_out, b_out))
    outs = [np.asarray(results[b]["out"]).reshape(C, 64, 64) for b in range(B)]
    return np.stack(outs, axis=0)


# revision 60
# speedup vs baseline: 1.1806x; 1.1153x over previous
"""Trainium2 Bass kernel for nn_CBSA_45389214384209 (sparse_attention).

Reference computation (per batch element b of 8):
  x_seq = x[b].T                      # [4096, 256]   (x[b] is [256, 4096])
  proj  = x_seq @ W_proj              # [4096, 512]
  rep   = avgpool8x8(proj)            # [64, 512]  == avgpool8x8(x_seq) @ W_proj
  per head h (8 heads, dh=64):
    S    = rep_h @ proj_h.T * scale   # [64, 4096]
    P    = softmax(S)                 # [64, 4096]
    rd   = P @ proj_h                 # [64, 64]
    rep2 = rep_h + step_rep[h] * rd
    P2   = softmax(rep2 @ rep2.T * scale)
    xd2  = step_x[h] * (P2 @ rep2)    # [64, 64]
    xdT  = xd2.T @ P                  # [64, 4096]  (back-projection, transposed)
  out[b] = W_out.T @ concat_h(xdT) + b_out[:, None]   # [256, 4096]

Sharding: pure data parallel - one batch element per NeuronCore (8 cores).

Layout strategy: projections are computed "transposed" (feature dim on
partitions) straight from the native DRAM layout of x; the final
x_outT[c, t] is exactly the DRAM layout of the output.  Head pairs are
packed on partitions (rows 0-63 = even head, 64-127 = odd head) with
block-diagonal stationary tiles so one 128-wide matmul serves two heads.
P^T (needed by the rep_delta contraction over tokens) is produced by the
DMA xbar transpose engine -- the PE never transposes the big P tiles.
rep_delta is accumulated in [query, dim] orientation so every softmax
normalization is a per-partition scalar multiply.  exp(S2) is symmetric,
which lets stage 2 skip the P2 transpose.  The output bias is folded into
the last matmul as a K=1 rank-1 update and the result is DMA'd to DRAM
directly from PSUM.
"""

import os
import sys

import numpy as np

for _p in ("/opt/trn_rl_repo", os.path.expanduser("~/.axon_site/_ro/trn_rl_repo")):
    if os.path.isdir(_p) and _p not in sys.path:
        sys.path.insert(0, _p)

import concourse.bass as bass
import concourse.tile as tile
from concourse import bacc, mybir
from concourse.bass import ds, ts
from concourse.masks import make_identity

F32 = mybir.dt.float32
F32R = mybir.dt.float32r
BF16 = mybir.dt.bfloat16
AX = mybir.AxisListType
ALU = mybir.AluOpType
ACTF = mybir.ActivationFunctionType

B = 8
C = 256          # model dim
T = 4096         # tokens (64x64 grid)
INNER = 512
HEADS = 8
DH = 64
NB = 64          # pooled tokens (8x8 grid)
SCALE = DH ** -0.5
NPAIR = 4        # head pairs
NCHUNK = 8       # 512-wide token chunks
NTT = 32         # 128-wide token tiles

CFG = {
    "dmat": True,             # DMA-xbar transpose for P^T (else PE transposes)
    "bf16_pass": False,       # bf16 projection passes (slower: swdge casts)
}


def build_module(cfg=CFG):
    nc = bacc.Bacc("TRN2", debug=False)

    io_dt = F32 if cfg["bf16_pass"] else F32R
    x = nc.dram_tensor("x", [C, T], io_dt, kind="ExternalInput").ap()
    wp = nc.dram_tensor("w_proj", [C, INNER], io_dt, kind="ExternalInput").ap()
    wo = nc.dram_tensor("w_out", [INNER, C], F32, kind="ExternalInput").ap()
    bo = nc.dram_tensor("b_out", [C], F32R, kind="ExternalInput").ap()
    srep = nc.dram_tensor("s_rep", [HEADS], F32, kind="ExternalInput").ap()
    sx = nc.dram_tensor("s_x", [HEADS], F32, kind="ExternalInput").ap()
    pmask = nc.dram_tensor(
        "pool_mask", [128, 8, NB], BF16, kind="ExternalInput"
    ).ap()
    out = nc.dram_tensor("out", [C, T], F32, kind="ExternalOutput").ap()

    with tile.TileContext(nc) as tc:
        _body(tc, cfg, x, wp, wo, bo, srep, sx, pmask, out)
    nc.compile()
    return nc


def _body(tc, cfg, x, wp, wo, bo, srep, sx, pmask, out):
    nc = tc.nc

    x_r = x.rearrange("(o p) t -> p o t", p=128)      # [128, 2, 4096]
    out_r = out.rearrange("(o p) t -> p o t", p=128)  # [128, 2, 4096]

    # ---- pools (SBUF pools stack-nested: alloc order == reverse release) --
    consts = tc.alloc_tile_pool(name="consts", bufs=1)
    stats = tc.alloc_tile_pool(name="stats", bufs=2)
    xdtp = tc.alloc_tile_pool(name="xdtp", bufs=2)       # xd2_bd + xdT chunks
    pp = tc.alloc_tile_pool(name="pp", bufs=1)           # P (attn) tiles
    ptp = tc.alloc_tile_pool(name="ptp", bufs=1)         # P^T tiles
    st2 = tc.alloc_tile_pool(name="st2", bufs=2)         # stage-2 temps
    prp = tc.alloc_tile_pool(name="prp", bufs=1)         # proj (bf16)
    pTp = tc.alloc_tile_pool(name="pTp", bufs=1)         # projT (bf16)
    xp = tc.alloc_tile_pool(name="xp", bufs=1)           # x (f32r)

    # PSUM tags: mm [128,2,512] x2 (4 banks) + outp x1 (2) + acc x1 + b3s x1.
    psum = tc.alloc_tile_pool(name="psum", bufs=1, space="PSUM")

    # ---- constants / weights (direct f32r DMA: same bits as f32) ---------
    # DMA issue order on the sync queue matters: wp + x chunks first (they
    # gate pass 1); everything needed later is issued after the x stream.
    ident_bf = consts.tile([128, 128], BF16, name="ident_bf")
    make_identity(nc, ident_bf)
    ident_f = consts.tile([128, 128], F32, name="ident_f")
    make_identity(nc, ident_f)

    P_DT = BF16 if cfg["bf16_pass"] else F32R
    wp_r = consts.tile([128, 2, INNER], P_DT, name="wp_r")
    if cfg["bf16_pass"]:
        nc.gpsimd.dma_start(wp_r, wp.rearrange("(o p) i -> p o i", p=128))
    else:
        nc.sync.dma_start(wp_r, wp.rearrange("(o p) i -> p o i", p=128))
    pmask_sb = consts.tile([128, 8, NB], BF16, name="pmask_sb")
    nc.sync.dma_start(pmask_sb, pmask)

    # ---- fused pass over x: projT (bf16) + proj (bf16), per 512-chunk ----
    # x streams through a 2-deep window; both projection orientations are
    # produced from the same resident chunk.
    projT = pTp.tile([128, 4, T], BF16, name="projT")
    proj_bf = prp.tile([128, NTT, INNER], BF16, name="proj_bf")
    # rep[q, d] = pool_mask^T @ proj accumulates on the PE inside the same
    # loop: pool_mask[:, br, :] has 1/64 at column (8*br + wb) for block-row
    # br; each 128-token tile lies in exactly one block-row (br = m//4).
    rep_pst = psum.tile([128, 512], F32, name="rep_pst", tag="outp", bufs=2)
    rep_ps = rep_pst[0:64, :]
    for j in range(NCHUNK):
        x_c = xp.tile([128, 2, 512], P_DT, name="x_c", tag="xc", bufs=2)
        if cfg["bf16_pass"]:
            nc.gpsimd.dma_start(x_c, x_r[:, :, ts(j, 512)])
        else:
            nc.sync.dma_start(x_c, x_r[:, :, ts(j, 512)])
        for gp in range(2):
            pt_ps = psum.tile([128, 2, 512], F32, name="pt_ps", tag="mm", bufs=2)
            for gi in range(2):
                g = 2 * gp + gi
                for o in range(2):
                    nc.tensor.matmul(
                        pt_ps[:, gi, :], wp_r[:, o, ts(g, 128)], x_c[:, o, :],
                        start=(o == 0), stop=(o == 1),
                    )
            if gp == 0:
                nc.scalar.copy(projT[:, ds(0, 2), ts(j, 512)], pt_ps)
            else:
                nc.vector.tensor_copy(projT[:, ds(2, 2), ts(j, 512)], pt_ps)
        for mp in range(2):
            pr_ps = psum.tile([128, 2, 512], F32, name="pr_ps", tag="mm", bufs=2)
            for mi in range(2):
                k = 2 * mp + mi
                for o in range(2):
                    nc.tensor.matmul(
                        pr_ps[:, mi, :], x_c[:, o, ts(k, 128)], wp_r[:, o, :],
                        start=(o == 0), stop=(o == 1),
                    )
            if mp == 0:
                nc.vector.tensor_copy(proj_bf[:, ds(4 * j, 2), :], pr_ps)
            else:
                nc.scalar.copy(proj_bf[:, ds(4 * j + 2, 2), :], pr_ps)
        # pool-MMs run one chunk behind so they never wait on the proj
        # copies of the chunk the PE just produced
        for k in range(4):
            if j == 0:
                break
            m = 4 * (j - 1) + k
            nc.tensor.matmul(
                rep_ps, pmask_sb[:, m // 4, :], proj_bf[:, m, :],
                start=(m == 0), stop=False,
            )
    for k in range(4):
        m = 4 * (NCHUNK - 1) + k
        nc.tensor.matmul(
            rep_ps, pmask_sb[:, m // 4, :], proj_bf[:, m, :],
            start=False, stop=(m == NTT - 1),
        )
    xp.release()

    # late-needed constants, issued behind the x stream
    wo_ld = consts.tile([128, 4, C], F32, name="wo_ld")
    nc.sync.dma_start(wo_ld, wo.rearrange("(g p) c -> p g c", p=128))
    wo_bf = consts.tile([128, 4, C], BF16, name="wo_bf")
    nc.vector.tensor_copy(wo_bf, wo_ld)
    bo_sb = consts.tile([1, 2, C // 2], F32R, name="bo_sb")
    nc.sync.dma_start(
        bo_sb,
        bass.AP(tensor=bo.tensor, offset=bo.offset,
                ap=[[0, 1], [C // 2, 2], [1, C // 2]]),
    )
    pmask_sb = consts.tile([128, 8, NB], BF16, name="pmask_sb")
    nc.sync.dma_start(pmask_sb, pmask)
    ones_f = consts.tile([1, 512], F32, name="ones_f")
    nc.vector.memset(ones_f, 1.0)
    ones_row = consts.tile([1, 512], F32R, name="ones_row")
    nc.vector.tensor_copy(ones_row, ones_f)

    # step_rep / step_x: column p holds step[2p] on partitions 0-63 and
    # step[2p+1] on partitions 64-127 (per-q-partition scalars).
    srep_ld = consts.tile([128, HEADS], F32, name="srep_ld")
    sx_ld = consts.tile([128, HEADS], F32, name="sx_ld")
    srep_bc = consts.tile([128, NPAIR], F32, name="srep_bc")
    sx_bc = consts.tile([128, NPAIR], F32, name="sx_bc")
    for st_dram, st_ld, st_bc in ((srep, srep_ld, srep_bc), (sx, sx_ld, sx_bc)):
        bcast = bass.AP(
            tensor=st_dram.tensor, offset=st_dram.offset,
            ap=[[0, 128], [st_dram.ap[0][0], HEADS]],
        )
        nc.sync.dma_start(st_ld, bcast)
        st_ldv = st_ld.rearrange("p (c two) -> p c two", two=2)
        for half in range(2):
            rows = slice(64 * half, 64 * half + 64)
            nc.vector.tensor_copy(st_bc[rows, :], st_ldv[rows, :, half])

    # ---- rep staging: repT_bd + rep_q from the pooled rep ----------------
    rep_sb = consts.tile([64, INNER], F32, name="rep_sb")
    nc.scalar.copy(rep_sb, rep_ps)
    # move a copy of rep onto partitions 64-127 (for the qB half of rep_q)
    qb_ps = psum.tile([128, 512], F32, name="qb_ps", tag="outp", bufs=2)
    nc.tensor.matmul(
        qb_ps[64:128, :], ident_f[0:64, 0:64], rep_sb, start=True, stop=True,
    )
    # repT_bd: scaled block-diagonal [[sA, 0], [0, sB]] per pair (one K=128
    # matmul computes both heads' scores; zeros kill the cross terms), and
    # rep_q[p] = [128 (qA|qB), 64 d] for the stage-2 update.
    repT_bd = consts.tile([128, NPAIR, 128], BF16, name="repT_bd")
    nc.vector.memset(repT_bd, 0.0)
    rep_q = consts.tile([128, NPAIR, NB], F32, name="rep_q")
    for g in range(4):
        rT_ps = psum.tile([128, NB], F32, name="rT_ps", tag="acc", bufs=1)
        nc.tensor.transpose(
            rT_ps, rep_sb[:, ds(128 * g, 128)], ident_f[0:64, 0:64]
        )
        for h in range(2):
            rows = slice(64 * h, 64 * h + 64)
            nc.vector.tensor_scalar_mul(
                repT_bd[rows, g, ds(64 * h, 64)], rT_ps[rows, :], SCALE
            )
        nc.vector.tensor_copy(rep_q[0:64, g, :], rep_sb[:, ds(128 * g, 64)])
        nc.vector.tensor_copy(
            rep_q[64:128, g, :], qb_ps[64:128, ds(128 * g + 64, 64)]
        )

    # ---- attention stages, pipelined per head pair ----------------------
    # P_sb[p]: [128, T]; rows 0-63 = head 2p, rows 64-127 = head 2p+1.
    # Scores are q.k/8 with pooled queries -> bounded |s| ~< 2, so softmax
    # needs no max-subtraction; exp reads score pairs straight from PSUM.
    # P stays unnormalized; 1/Z folds into rep_delta (per-partition) and
    # into the back-projection stationary (rz*rz2*step_x on xd2).
    p_tiles = [
        pp.tile([128, T], BF16, name=f"p{p}", tag=f"p{p}") for p in range(NPAIR)
    ]
    xd2_tiles = []
    xdt_tiles = [[] for _ in range(NPAIR)]
    pt_list = [None] * NPAIR
    rz_list = [None] * NPAIR

    def _score_phase(p):
        # S + exp + P^T for pair p.  Sub-DMA-transposes chase the exp stream
        # so pt is ready ~one group after the last exp, not 3.6us later.
        zpart = stats.tile([128, 4], F32, name="zpart", tag="zpart")
        pt_t = ptp.tile([128, NTT, 128], BF16, name=f"pt{p}", tag="pt", bufs=4)
        for jp in range(4):
            s_ps = psum.tile([128, 2, 512], F32, name="s_ps", tag="mm", bufs=2)
            for ji in range(2):
                nc.tensor.matmul(
                    s_ps[:, ji, :], repT_bd[:, p, :],
                    projT[:, p, ts(2 * jp + ji, 512)],
                    start=True, stop=True,
                )
            nc.scalar.activation(
                out=p_tiles[p][:, ts(jp, 1024)].rearrange(
                    "p (a b) -> p a b", a=2),
                in_=s_ps, func=ACTF.Exp,
                bias=0.0, scale=1.0, accum_out=zpart[:, jp : jp + 1],
            )
            if cfg["dmat"]:
                nc.sync.dma_start_transpose(
                    pt_t[:, ds(8 * jp, 8), :],
                    p_tiles[p][:, ts(jp, 1024)],
                )
            else:
                for mq in range(2):
                    tp_ps = psum.tile(
                        [128, 4, 128], BF16, name="tp_ps", tag="b3s", bufs=1)
                    for c4 in range(4):
                        nc.tensor.transpose(
                            tp_ps[:, c4, :],
                            p_tiles[p][:, ts(8 * jp + 4 * mq + c4, 128)],
                            ident_bf)
                    nc.vector.tensor_copy(
                        pt_t[:, ds(8 * jp + 4 * mq, 4), :], tp_ps)
        zsum = stats.tile([128, 1], F32, name="zsum", tag="zsum")
        nc.vector.reduce_sum(zsum, zpart, axis=AX.X)
        rz = stats.tile([128, 1], F32, name="rz", tag=f"rzp{p}", bufs=1)
        nc.vector.reciprocal(rz, zsum)
        pt_list[p] = pt_t
        rz_list[p] = rz
        if p == NPAIR - 1:
            pTp.release()

    def _delta_phase(p):
        rz = rz_list[p]
        # rep_delta [q, d]: av[q, d'] = sum_t P[q,t] proj[t,d'] accumulated
        # over 32 token tiles; diagonal 64x64 blocks are the two heads'.
        av_ps = psum.tile([128, 128], F32, name="av_ps", tag="acc", bufs=1)
        for m in range(NTT):
            nc.tensor.matmul(
                av_ps, pt_list[p][:, m, :], proj_bf[:, m, ds(128 * p, 128)],
                start=(m == 0), stop=(m == NTT - 1),
            )
        # rep2 = rep_q + (step_rep * rz) * rep_delta   [128 q-pair, 64 d]
        srz = stats.tile([128, 1], F32, name="srz", tag="srz")
        nc.vector.tensor_mul(srz, rz, srep_bc[:, p : p + 1])
        rep2 = st2.tile([128, NB], BF16, name="rep2", tag="rep2")
        for h in range(2):
            rows = slice(64 * h, 64 * h + 64)
            nc.vector.scalar_tensor_tensor(
                rep2[rows, :], av_ps[rows, ds(64 * h, 64)], srz[rows, :],
                rep_q[rows, p, :], op0=ALU.mult, op1=ALU.add,
            )
        # rep2T block-diag (zeros kill cross-head terms in S2)
        r2t_ps = psum.tile([64, 128], BF16, name="r2t_ps", tag="b3s", bufs=1)
        nc.tensor.transpose(r2t_ps, rep2, ident_bf)
        r2bd = st2.tile([128, 128], BF16, name="r2bd", tag="r2bd")
        nc.vector.memset(r2bd, 0.0)
        for h in range(2):
            nc.vector.tensor_copy(
                r2bd[ds(64 * h, 64), ds(64 * h, 64)],
                r2t_ps[:, ds(64 * h, 64)],
            )
        # S2 = rep2 @ rep2^T (both heads at once); exp(SCALE*S2) on the
        # diagonal quadrants only (cross quadrants stay zero).  exp(S2) is
        # symmetric, so it serves directly as lhsT for xd2 = P2 @ rep2.
        s2_ps = psum.tile([128, 128], F32, name="s2_ps", tag="b3s", bufs=1)
        nc.tensor.matmul(s2_ps, r2bd, r2bd, start=True, stop=True)
        es = st2.tile([128, 128], BF16, name="es", tag="es")
        nc.vector.memset(es, 0.0)
        z2 = stats.tile([128, 1], F32, name="z2", tag="z2")
        for h in range(2):
            rows = slice(64 * h, 64 * h + 64)
            nc.scalar.activation(
                out=es[rows, ds(64 * h, 64)], in_=s2_ps[rows, ds(64 * h, 64)],
                func=ACTF.Exp, bias=0.0, scale=SCALE,
                accum_out=z2[rows, :],
            )
        rz2 = stats.tile([128, 1], F32, name="rz2", tag="rz2")
        nc.vector.reciprocal(rz2, z2)
        xd_ps = psum.tile([128, NB], F32, name="xd_ps", tag="b3s", bufs=1)
        nc.tensor.matmul(xd_ps, es, rep2, start=True, stop=True)
        # xd2_bd: block-diagonal stationary for the back-projection, with
        # rz (stage-1 softmax), rz2 (stage-2 softmax) and step_x folded in.
        rzsx = stats.tile([128, 1], F32, name="rzsx", tag="rzsx")
        nc.vector.tensor_mul(rzsx, rz, sx_bc[:, p : p + 1])
        nc.vector.tensor_mul(rzsx, rzsx, rz2)
        xd2_bd = xdtp.tile([128, 128], BF16, name="xd2_bd", tag=f"xd2_{p}",
                           bufs=1)
        nc.vector.memset(xd2_bd, 0.0)
        for h in range(2):
            rows = slice(64 * h, 64 * h + 64)
            nc.vector.tensor_scalar_mul(
                xd2_bd[rows, ds(64 * h, 64)], xd_ps[rows, :], rzsx[rows, :],
            )
        xd2_tiles.append(xd2_bd)

    def _bp_phase(pq):
        # back-projection for a pair-pair; pq=0 overlaps pair-3 attention.
        # pi-major order: each xd2 stationary is loaded once and streams 8
        # matmuls.
        for pi in range(2):
            for j2 in range(NCHUNK // 2):
                bp_ps = psum.tile(
                    [128, 2, 512], F32, name="bp_ps", tag="mm", bufs=2)
                for ji in range(2):
                    nc.tensor.matmul(
                        bp_ps[:, ji, :], xd2_tiles[2 * pq + pi],
                        p_tiles[2 * pq + pi][:, ts(2 * j2 + ji, 512)],
                        start=True, stop=True,
                    )
                xdt_sb = xdtp.tile(
                    [128, 2, 512], BF16, name="xdt_sb",
                    tag=f"xdt_{pq}_{pi}_{j2}", bufs=1)
                if j2 % 2 == 0:
                    nc.vector.tensor_copy(xdt_sb, bp_ps)
                else:
                    nc.scalar.copy(xdt_sb, bp_ps)
                xdt_tiles[2 * pq + pi].append(xdt_sb)

    # software pipeline with a 1-pair offset: the PE fills the wait for
    # pair p's P^T (exp + DMA transpose) with pair p-1's delta/stage-2
    # work; back-projections are emitted one step after their xd2 tiles
    # complete so the in-order PE queue never waits on a stage-2 chain.
    for step in range(NPAIR + 2):
        if step < NPAIR:
            _score_phase(step)
        if 1 <= step <= NPAIR:
            _delta_phase(step - 1)
        if step >= 3 and (step - 3) % 2 == 0:
            _bp_phase((step - 3) // 2)
    prp.release()
    st2.release()
    ptp.release()
    pp.release()

    # ---- output projection, per (chunk, 128-channel group) ---------------
    for j in range(NCHUNK):
        for ct in range(2):
            op_ps = psum.tile([128, 512], F32, name="op_ps", tag="outp",
                              bufs=2)
            for g in range(4):
                nc.tensor.matmul(
                    op_ps, wo_bf[:, g, ts(ct, 128)],
                    xdt_tiles[g][j // 2][:, j % 2, :],
                    start=(g == 0), stop=False,
                )
            nc.tensor.matmul(
                op_ps, bo_sb[:, ct, :], ones_row, start=False, stop=True,
            )
            out_sb = xdtp.tile([128, 512], F32, name="out_sb", tag="out_sb")
            if (2 * j + ct) % 2 == 0:
                nc.scalar.copy(out_sb, op_ps)
            else:
                nc.vector.tensor_copy(out_sb, op_ps)
            nc.sync.dma_start(out_r[:, ct, ts(j, 512)], out_sb)
    xdtp.release()
    psum.release()
    stats.release()
    consts.release()


_CACHE = {}


class _Runner:
    """Builds the Bass module once and keeps a single jitted shard_map
    executable alive, so repeat kernel() calls skip retracing/relowering."""

    def __init__(self):
        import jax
        import jax.numpy as jnp
        from jax.sharding import Mesh, PartitionSpec
        from jax.experimental.shard_map import shard_map
        from concourse import bass2jax

        self.jax = jax
        nc = build_module()
        self.nc = nc
        bass2jax.install_neuronx_cc_hook()

        partition_name = (
            nc.partition_id_tensor.name if nc.partition_id_tensor else None
        )
        in_names, out_names, out_avals = [], [], []
        for alloc in nc.m.functions[0].allocations:
            if not isinstance(alloc, mybir.MemoryLocationSet):
                continue
            name = alloc.memorylocations[0].name
            if alloc.kind == "ExternalInput":
                if name != partition_name:
                    in_names.append(name)
            elif alloc.kind == "ExternalOutput":
                out_names.append(name)
                out_avals.append(
                    jax.core.ShapedArray(
                        tuple(alloc.tensor_shape), mybir.dt.np(alloc.dtype)
                    )
                )
        n_params = len(in_names)
        n_outs = len(out_avals)
        all_names = list(in_names) + list(out_names)
        if partition_name is not None:
            all_names.append(partition_name)
        self.in_names = in_names
        self.out_names = out_names
        self.out_avals = out_avals

        def _body(*args):
            operands = list(args)
            if partition_name is not None:
                operands.append(bass2jax.partition_id_tensor())
            outs = bass2jax._bass_exec_p.bind(
                *operands,
                out_avals=tuple(out_avals),
                in_names=tuple(all_names),
                out_names=tuple(out_names),
                lowering_input_output_aliases=(),
                sim_require_finite=True,
                sim_require_nnan=True,
                nc=nc,
            )
            return tuple(outs)

        self.body = _body
        devices = jax.devices()[:B]
        mesh = Mesh(np.asarray(devices), ("core",))
        donate = tuple(range(n_params, n_params + n_outs))
        self.sharded = jax.jit(
            shard_map(
                _body, mesh=mesh,
                in_specs=(PartitionSpec("core"),) * (n_params + n_outs),
                out_specs=(PartitionSpec("core"),) * n_outs,
                check_rep=False,
            ),
            donate_argnums=donate,
            keep_unused=True,
        )

    def run(self, in_maps):
        concat_in = [
            np.concatenate([m[name] for m in in_maps], axis=0)
            for name in self.in_names
        ]
        zeros = [
            np.zeros((B * a.shape[0], *a.shape[1:]), a.dtype) for a in self.out_avals
        ]
        out_arrs = self.sharded(*concat_in, *zeros)
        return [
            {
                name: np.asarray(out_arrs[i]).reshape(B, *self.out_avals[i].shape)[c]
                for i, name in enumerate(self.out_names)
            }
            for c in range(B)
        ]

    def bench(self, in_maps, reps=8, inner=72, base=8):
        """Time device-resident executions (no donation, operands staged once).

        Times jitted chains of `base` and `inner` back-to-back kernel
        executions; returns (per_exec_seconds, base_chain_seconds, results)
        with per_exec = (t_inner - t_base) / (inner - base), which amortizes
        away the per-dispatch round-trip of this axon-tunneled environment.
        """
        import time
        from jax.sharding import Mesh, PartitionSpec, NamedSharding
        from jax.experimental.shard_map import shard_map

        jax = self.jax
        devices = jax.devices()[:B]
        mesh = Mesh(np.asarray(devices), ("core",))
        sharding = NamedSharding(mesh, PartitionSpec("core"))
        n_ops = len(self.in_names) + len(self.out_avals)

        def chain(n):
            def f(*args):
                outs = []
                for _ in range(n):
                    outs.extend(self.body(*args))
                return tuple(outs)
            return f

        concat_in = [
            np.concatenate([m[name] for m in in_maps], axis=0)
            for name in self.in_names
        ]
        zeros = [
            np.zeros((B * a.shape[0], *a.shape[1:]), a.dtype) for a in self.out_avals
        ]
        staged = [jax.device_put(a, sharding) for a in concat_in + zeros]

        times = {}
        out1 = None
        for n in (base, inner):
            jfn = jax.jit(
                shard_map(
                    chain(n), mesh=mesh,
                    in_specs=(PartitionSpec("core"),) * n_ops,
                    out_specs=(PartitionSpec("core"),) * (n * len(self.out_avals)),
                    check_rep=False,
                ),
                keep_unused=True,
            )
            out = jfn(*staged)
            jax.block_until_ready(out)
            best = float("inf")
            for _ in range(reps):
                t0 = time.perf_counter()
                out = jfn(*staged)
                jax.block_until_ready(out)
                best = min(best, time.perf_counter() - t0)
            times[n] = best
            if n == base:
                out1 = out
        per_exec = (times[inner] - times[base]) / (inner - base)
        if per_exec <= 0:
            per_exec = times[inner] / inner  # noise floor: report upper bound
        results = [
            {
                name: np.asarray(out1[i]).reshape(B, *self.out_avals[i].shape)[c]
                for i, name in enumerate(self.out_names)
            }
            for c in range(B)
        ]
        return per_exec, times[base], results


def _get_runner():
    key = tuple(sorted(CFG.items()))
    if key not in _CACHE:
        _CACHE[key] = _Runner()
    return _CACHE[key]


def _pool_mask():
    import ml_dtypes

    mask = np.zeros((128, 8, NB), dtype=np.float32)
    for tp in range(128):
        wb = (tp % 64) // 8
        for br in range(8):
            mask[tp, br, 8 * br + wb] = 1.0 / 64.0
    return mask.astype(ml_dtypes.bfloat16)


def _make_in_maps(x, W_proj, step_rep, step_x, W_out, b_out):
    x = np.ascontiguousarray(np.asarray(x, dtype=np.float32))
    shared = {
        "w_proj": np.ascontiguousarray(np.asarray(W_proj, dtype=np.float32)),
        "w_out": np.ascontiguousarray(np.asarray(W_out, dtype=np.float32)),
        "b_out": np.ascontiguousarray(np.asarray(b_out, dtype=np.float32)),
        "s_rep": np.ascontiguousarray(
            np.asarray(step_rep, dtype=np.float32).reshape(HEADS)
        ),
        "s_x": np.ascontiguousarray(
            np.asarray(step_x, dtype=np.float32).reshape(HEADS)
        ),
        "pool_mask": _pool_mask(),
    }
    return [
        {"x": np.ascontiguousarray(x[b].reshape(C, T)), **shared} for b in range(B)
    ]


def kernel(x, W_proj, step_rep, step_x, W_out, b_out):
    runner = _get_runner()
    results = runner.run(_make_in_maps(x, W_proj, step_rep, step_x, W_out, b_out))
    outs = [np.asarray(results[b]["out"]).reshape(C, 64, 64) for b in range(B)]
    return np.stack(outs, axis=0)
